# revision 1
# baseline (speedup 1.0000x reference)
"""Trainium2 Bass kernel for a 2-layer edge-featured GAT + mean-pool + FC.

Sharding: 256 graphs are split 32-per-core across 8 cores. Because `batch` is
sorted, each core owns a contiguous node range (graph-aligned), so both the
per-destination softmax segments and the mean-pool segments are core-local.
Edges are assigned to the core that owns their destination node. Between
layers, per-core node tables (features + attention logit terms) are
AllGathered so every core can gather arbitrary source rows.

Edge phase (per layer): edges sorted by dst are packed into 128-edge chunks
grouped by destination node tile (128 nodes). Per chunk, a dma_gather pulls
the source-node table rows [h | a_src | a_dst | pad], a second (narrow)
dma_gather pulls the destination rows' logit slice. Attention weights are
p = exp(leaky_relu(a_src+a_dst+w*q)) computed as max(exp(x), exp(0.2x)),
messages are h*p, and the segment-sum over destinations is a one-hot matmul
accumulated in PSUM — with p itself carried as extra columns to produce the
softmax denominators. Softmax normalization happens once per node after
aggregation: out = (sum p*h) / (sum p + 1e-16), exactly equivalent to the
reference's per-edge normalization (the max-subtraction cancels in the
ratio). Pad edge slots carry dst_local = -1 so their one-hot columns are all
zero and they contribute nothing.
"""

import sys

sys.path.insert(0, "/opt/trn_rl_repo")

import math
from contextlib import ExitStack

import numpy as np

import concourse.bacc as bacc
import concourse.bass as bass
import concourse.mybir as mybir
import concourse.tile as tile
from concourse.bass_utils import run_bass_kernel_spmd
from concourse.masks import make_identity

P = 128
NCORES = 8
SP = False  # dma_gather single_packet
SKIP = set()  # benchmarking ablations: gsrc, gdst, msg, mm

FULL_CFG = dict(N=20000, E=640000, FIN=128, HID=64, HEADS=4, NG=256, OUT=32)

F32 = mybir.dt.float32
I16 = mybir.dt.int16


# ---------------------------------------------------------------------------
# Host-side preparation: integer index manipulation + array reordering only.
# ---------------------------------------------------------------------------
def prepare(inputs, cfg):
    N, E, FIN, HID, HEADS, NG, OUT = (
        cfg["N"], cfg["E"], cfg["FIN"], cfg["HID"], cfg["HEADS"], cfg["NG"],
        cfg["OUT"],
    )
    GPC = NG // NCORES  # graphs per core

    x = np.asarray(inputs["x"], np.float32)
    ei = np.asarray(inputs["edge_index"], np.int64)
    ea = np.asarray(inputs["edge_attr"], np.float32)
    batch = np.asarray(inputs["batch"], np.int64)
    src, dst = ei[0], ei[1]

    # node ranges per core (graph-aligned; batch is sorted)
    bounds = np.searchsorted(batch, np.arange(NCORES + 1) * GPC)
    node_cnt = np.diff(bounds)
    NT = max(1, math.ceil(node_cnt.max() / P))
    NSLICE = NT * P
    NROWS = NCORES * NSLICE
    assert NROWS < 32768, f"int16 gather index overflow: {NROWS}"

    core_of_node = np.minimum(batch // GPC, NCORES - 1).astype(np.int64)
    rowid = np.empty(N, np.int64)
    for c in range(NCORES):
        ns, ne = bounds[c], bounds[c + 1]
        rowid[ns:ne] = c * NSLICE + np.arange(ne - ns)

    # edges sorted by dst; since batch is sorted, core blocks are contiguous
    order = np.argsort(dst, kind="stable")
    dsts = dst[order]
    srcs = src[order]
    ws = ea[order, 0]
    ecore = core_of_node[dsts]
    ebounds = np.searchsorted(ecore, np.arange(NCORES + 1))

    # chunks-per-tile: max over all (core, tile), rounded up to even
    cpt_max = 1
    tile_edge_counts = []
    for c in range(NCORES):
        es, ee = ebounds[c], ebounds[c + 1]
        dln = dsts[es:ee] - bounds[c]
        tid = dln // P
        cnts = np.bincount(tid, minlength=NT)
        tile_edge_counts.append(cnts)
        if len(cnts):
            cpt_max = max(cpt_max, math.ceil(cnts.max() / P))
    CPT = cpt_max + (cpt_max % 2)  # even
    CPT = max(CPT, 2)
    CH = CPT // 2
    NCHUNK = NT * CPT

    per_core = []
    for c in range(NCORES):
        ns, ne = bounds[c], bounds[c + 1]
        es, ee = ebounds[c], ebounds[c + 1]
        nloc = ne - ns

        xs = np.zeros((NSLICE, FIN), np.float32)
        xs[:nloc] = x[ns:ne]

        gl = np.full((NT * P,), -1.0, np.float32)
        gl[:nloc] = (batch[ns:ne] - c * GPC).astype(np.float32)
        gl_dev = gl.reshape(NT, P).T.copy()  # [128, NT]

        srcrow = np.zeros((NT, CPT * P), np.int64)
        dstrow = np.zeros((NT, CPT * P), np.int64)
        dstl = np.full((NT, CPT * P), -1.0, np.float32)
        wv = np.zeros((NT, CPT * P), np.float32)

        dln = dsts[es:ee] - ns
        tid = dln // P
        cnts = tile_edge_counts[c]
        off = np.zeros(NT + 1, np.int64)
        off[1:NT + 1] = np.cumsum(cnts[:NT])
        for t in range(NT):
            k = int(cnts[t]) if t < len(cnts) else 0
            if k == 0:
                continue
            sel = slice(es + int(off[t]), es + int(off[t]) + k)
            srcrow[t, :k] = rowid[srcs[sel]]
            dstrow[t, :k] = rowid[dsts[sel]]
            dstl[t, :k] = (dln[int(off[t]):int(off[t]) + k] % P).astype(
                np.float32)
            wv[t, :k] = ws[sel]

        # device layouts
        dstl_dev = dstl.reshape(NCHUNK, P).T.copy()       # [128, NCHUNK]
        wv_dev = wv.reshape(NCHUNK, P).T.copy()

        def wrap_idx(arr):  # [NT, CPT*P] -> [128, NT*CPT*8] int16
            blocks = []
            for t in range(NT):
                for h in range(2):
                    ids = arr[t, h * CH * P:(h + 1) * CH * P]
                    a = ids.reshape(CH * 8, 16).T  # [16, CH*8]
                    blocks.append(np.tile(a, (8, 1)))
            return np.ascontiguousarray(
                np.concatenate(blocks, axis=1)).astype(np.int16)

        per_core.append(dict(
            xs=xs, gl=gl_dev, dstl=dstl_dev, wv=wv_dev,
            idxs=wrap_idx(srcrow), idxd=wrap_idx(dstrow),
        ))

    # weight-side constants (tiny, host-replicated)
    W1 = np.asarray(inputs["W1"], np.float32)            # [FIN, H*HID]
    W2 = np.asarray(inputs["W2"], np.float32)            # [H*HID, HID]
    as1 = np.asarray(inputs["att_src1"], np.float32).reshape(-1)
    ad1 = np.asarray(inputs["att_dst1"], np.float32).reshape(-1)
    as2 = np.asarray(inputs["att_src2"], np.float32).reshape(-1)
    ad2 = np.asarray(inputs["att_dst2"], np.float32).reshape(-1)
    q1 = (np.asarray(inputs["We1"], np.float32).reshape(HEADS, HID)
          * np.asarray(inputs["att_edge1"], np.float32)).sum(axis=1)  # [H]
    q2 = float((np.asarray(inputs["We2"], np.float32).reshape(-1)
                * np.asarray(inputs["att_edge2"], np.float32).reshape(-1))
               .sum())
    b1 = np.asarray(inputs["b1"], np.float32)
    b2 = np.asarray(inputs["b2"], np.float32)
    fcW = np.asarray(inputs["fcW"], np.float32)
    fcb = np.asarray(inputs["fcb"], np.float32)

    rep = lambda vv: np.tile(vv[None, :].astype(np.float32), (P, 1)).copy()
    consts = dict(
        W1=W1, W2=W2,
        as1b=rep(as1), ad1b=rep(ad1), b1b=rep(b1),
        as2b=rep(as2), ad2b=rep(ad2), b2b=rep(b2),
        q1b=rep(q1), q2b=np.full((P, 1), q2, np.float32),
        fcw=fcW, fcbb=rep(fcb),
        iota=np.tile(np.arange(P, dtype=np.float32)[None, :], (P, 1)).copy(),
    )

    in_maps = []
    for c in range(NCORES):
        m = dict(per_core[c])
        m.update(consts)
        in_maps.append(m)

    meta = dict(NT=NT, CPT=CPT, CH=CH, NSLICE=NSLICE, NROWS=NROWS,
                GPC=GPC, **cfg)
    return in_maps, meta


# ---------------------------------------------------------------------------
# Device program.
# ---------------------------------------------------------------------------
def build(meta, reps=1, num_devices=NCORES):
    NT, CPT, CH = meta["NT"], meta["CPT"], meta["CH"]
    NSLICE, NROWS, GPC = meta["NSLICE"], meta["NROWS"], meta["GPC"]
    FIN, HID, HEADS, OUT = meta["FIN"], meta["HID"], meta["HEADS"], meta["OUT"]
    D1 = HEADS * HID          # 256
    ROW1 = D1 + 64            # 320 floats: h(256) asrc(4@256) adst(4@260) pad
    OFF1 = D1                 # dst-slice offset (floats)
    ROW2 = 2 * HID            # 128 floats: h2(64) asrc2(@64) adst2(@65) pad
    OFF2 = HID
    NI = CH * P               # idxs per gather group
    NIc = NI // 16            # idx columns per group
    NCHUNK = NT * CPT
    A = mybir.AluOpType
    ACT = mybir.ActivationFunctionType
    X = mybir.AxisListType.X
    rg = [list(range(NCORES))]

    nc = bacc.Bacc("TRN2", target_bir_lowering=False, debug=False,
                   num_devices=num_devices,
                   dynamic_dma_scratch_size=65536)

    def din(name, shape, dtype=F32):
        return nc.dram_tensor(name, list(shape), dtype,
                              kind="ExternalInput").ap()

    xs = din("xs", (NSLICE, FIN))
    idxs_d = din("idxs", (P, NT * CPT * 8), I16)
    idxd_d = din("idxd", (P, NT * CPT * 8), I16)
    dstl_d = din("dstl", (P, NCHUNK))
    wv_d = din("wv", (P, NCHUNK))
    gl_d = din("gl", (P, NT))
    W1_d = din("W1", (FIN, D1))
    W2_d = din("W2", (D1, HID))
    as1_d = din("as1b", (P, D1))
    ad1_d = din("ad1b", (P, D1))
    b1_d = din("b1b", (P, D1))
    as2_d = din("as2b", (P, HID))
    ad2_d = din("ad2b", (P, HID))
    b2_d = din("b2b", (P, HID))
    q1_d = din("q1b", (P, HEADS))
    q2_d = din("q2b", (P, 1))
    fcw_d = din("fcw", (HID, OUT))
    fcb_d = din("fcbb", (P, OUT))
    iota_d = din("iota", (P, P))

    out_d = nc.dram_tensor("out", [GPC, OUT], F32, kind="ExternalOutput").ap()

    with tile.TileContext(nc) as tc, ExitStack() as st:
        constp = st.enter_context(tc.tile_pool(name="constp", bufs=1))
        drp = st.enter_context(tc.tile_pool(name="drp", bufs=1, space="DRAM"))

        # whole-kernel constants
        iota_sb = constp.tile([P, P], F32)
        nc.sync.dma_start(iota_sb[:], iota_d[:])
        ident = constp.tile([P, P], F32)
        make_identity(nc, ident[:])
        dstl_sb = constp.tile([P, NCHUNK], F32)
        nc.sync.dma_start(dstl_sb[:], dstl_d[:])
        wv_sb = constp.tile([P, NCHUNK], F32)
        nc.sync.dma_start(wv_sb[:], wv_d[:])
        gl_sb = constp.tile([P, NT], F32)
        nc.sync.dma_start(gl_sb[:], gl_d[:])
        q1_sb = constp.tile([P, HEADS], F32)
        nc.sync.dma_start(q1_sb[:], q1_d[:])
        q2_sb = constp.tile([P, 1], F32)
        nc.sync.dma_start(q2_sb[:], q2_d[:])
        ixs_all = constp.tile([P, NT * CPT * 8], I16)
        nc.sync.dma_start(ixs_all[:], idxs_d[:])
        ixd_all = constp.tile([P, NT * CPT * 8], I16)
        nc.sync.dma_start(ixd_all[:], idxd_d[:])

        # repetition loop (reps>1 only for benchmarking)
        for _rep in range(reps):
            t1loc = drp.tile([NSLICE, ROW1], F32, name=f"t1loc{_rep}")
            t1full = drp.tile([NROWS, ROW1], F32, addr_space="Shared",
                              name=f"t1full{_rep}")
            t2loc = drp.tile([NSLICE, ROW2], F32, name=f"t2loc{_rep}")
            t2full = drp.tile([NROWS, ROW2], F32, addr_space="Shared",
                              name=f"t2full{_rep}")

            # ---------------- Phase 0: h1 = x @ W1, a_src/a_dst, table1 ---------
            with tc.tile_pool(name="ph0", bufs=1) as sp, \
                 tc.tile_pool(name="ph0b", bufs=2) as sp2, \
                 tc.tile_pool(name="ph0p", bufs=2, space="PSUM") as pp:
                w1_sb = sp.tile([P, D1], F32)
                nc.sync.dma_start(w1_sb[:], W1_d[:])
                as1_sb = sp.tile([P, D1], F32)
                nc.sync.dma_start(as1_sb[:], as1_d[:])
                ad1_sb = sp.tile([P, D1], F32)
                nc.sync.dma_start(ad1_sb[:], ad1_d[:])
                xall = sp.tile([P, NT, FIN], F32)
                nc.sync.dma_start(xall[:],
                                  xs[:].rearrange("(t p) f -> p t f", p=P))
                for t in range(NT if "ph0" not in SKIP else 0):
                    xT_ps = pp.tile([P, P], F32, space="PSUM")
                    nc.tensor.transpose(xT_ps[:], xall[:, t, :], ident[:])
                    xT = sp2.tile([P, P], F32)
                    nc.vector.tensor_copy(out=xT[:], in_=xT_ps[:])
                    h_ps = pp.tile([P, D1], F32, space="PSUM")
                    nc.tensor.matmul(h_ps[:], lhsT=xT[:], rhs=w1_sb[:],
                                     start=True, stop=True)
                    t1t = sp2.tile([P, ROW1], F32)
                    tmp = sp2.tile([P, D1], F32)
                    nc.vector.tensor_tensor(out=tmp[:], in0=h_ps[:],
                                            in1=as1_sb[:], op=A.mult)
                    nc.vector.tensor_reduce(
                        out=t1t[:, D1:D1 + HEADS],
                        in_=tmp[:].rearrange("p (h f) -> p h f", h=HEADS),
                        axis=X, op=A.add)
                    nc.vector.tensor_tensor(out=tmp[:], in0=h_ps[:],
                                            in1=ad1_sb[:], op=A.mult)
                    nc.vector.tensor_reduce(
                        out=t1t[:, D1 + HEADS:D1 + 2 * HEADS],
                        in_=tmp[:].rearrange("p (h f) -> p h f", h=HEADS),
                        axis=X, op=A.add)
                    nc.vector.tensor_copy(out=t1t[:, 0:D1], in_=h_ps[:])
                    nc.vector.memset(t1t[:, D1 + 2 * HEADS:ROW1], 0.0)
                    nc.sync.dma_start(t1loc[t * P:(t + 1) * P, :], t1t[:])
                if "ag" not in SKIP:
                    nc.gpsimd.collective_compute(
                        "AllGather", A.bypass, replica_groups=rg,
                        ins=[t1loc[:]], outs=[t1full[:]])

            # ---------------- Phase 1: layer-1 edge phase -----------------------
            with tc.tile_pool(name=f"outp{_rep}", bufs=1) as outp:
              out1 = outp.tile([P, NT * D1], F32, name=f"out1_{_rep}")
              if "ph1" in SKIP:
                  nc.vector.memset(out1[:], 0.0)
              with tc.tile_pool(name="p1g", bufs=2) as pg, \
                   tc.tile_pool(name="p1gd", bufs=2) as pgd, \
                   tc.tile_pool(name="p1i", bufs=3) as pi, \
                   tc.tile_pool(name="p1w", bufs=2) as pw, \
                   tc.tile_pool(name="p1oh", bufs=2) as poh, \
                   tc.tile_pool(name="p1ps", bufs=3, space="PSUM") as pps:
                  for t in range(NT if "ph1" not in SKIP else 0):
                      acc = pps.tile([P, D1 + HEADS], F32, space="PSUM")
                      for hh in range(2):
                          gbase = (t * 2 + hh) * NIc
                          cbase = (t * 2 + hh) * CH
                          G = pg.tile([P, CH, ROW1], F32)
                          if "gsrc" not in SKIP:
                              nc.gpsimd.dma_gather(
                                  G[:], t1full[:],
                                  ixs_all[:, gbase:gbase + NIc],
                                  NI, NI, ROW1, single_packet=SP)
                          Gd = pgd.tile([P, CH, 64], F32)
                          asr = G[:, :, D1:D1 + HEADS]
                          if "gdst" not in SKIP:
                              nc.gpsimd.dma_gather(
                                  Gd[:], t1full[:, OFF1:OFF1 + 64],
                                  ixd_all[:, gbase:gbase + NIc],
                                  NI, NI, 64, elem_step=ROW1, single_packet=SP)
                              nc.vector.tensor_tensor(
                                  out=asr, in0=asr,
                                  in1=Gd[:, :, HEADS:2 * HEADS], op=A.add)
                          if "msg" not in SKIP:
                              ae = pw.tile([P, CH, HEADS], F32)
                              w_b = wv_sb[:, cbase:cbase + CH].unsqueeze(2) \
                                  .to_broadcast([P, CH, HEADS])
                              q_b = q1_sb[:].unsqueeze(1).to_broadcast([P, CH, HEADS])
                              nc.vector.tensor_tensor(out=ae[:], in0=w_b, in1=q_b,
                                                      op=A.mult)
                              nc.vector.tensor_tensor(out=asr, in0=asr, in1=ae[:],
                                                      op=A.add)
                              e2 = pw.tile([P, CH, HEADS], F32)
                              nc.scalar.activation(out=e2[:], in_=asr, func=ACT.Exp,
                                                   scale=0.2)
                              nc.scalar.activation(out=asr, in_=asr, func=ACT.Exp)
                              nc.vector.tensor_tensor(out=asr, in0=asr, in1=e2[:],
                                                      op=A.max)
                              gm = G[:, :, 0:D1].rearrange("p c (h f) -> p c h f",
                                                               h=HEADS)
                              p_b = asr.unsqueeze(3).to_broadcast([P, CH, HEADS, HID])
                              nc.vector.tensor_tensor(out=gm, in0=gm, in1=p_b,
                                                      op=A.mult)
                          if "mm" not in SKIP:
                              oh = poh.tile([P, CH, P], F32)
                              nc.vector.tensor_tensor(
                                  out=oh[:],
                                  in0=iota_sb[:].unsqueeze(1)
                                      .to_broadcast([P, CH, P]),
                                  in1=dstl_sb[:, cbase:cbase + CH]
                                      .unsqueeze(2).to_broadcast([P, CH, P]),
                                  op=A.is_equal)
                              for c in range(CH):
                                  nc.tensor.matmul(
                                      acc[:], lhsT=oh[:, c, :],
                                      rhs=G[:, c, 0:D1 + HEADS],
                                      start=(hh == 0 and c == 0),
                                      stop=(hh == 1 and c == CH - 1))
                      if "epi" not in SKIP:
                          # epilogue -> out1 tile block (softmax denominator division)
                          dn = pw.tile([P, HEADS], F32)
                          nc.vector.tensor_scalar(out=dn[:], in0=acc[:, D1:D1 + HEADS],
                                                  scalar1=1e-16, scalar2=None,
                                                  op0=A.add)
                          rc = pw.tile([P, HEADS], F32)
                          nc.vector.reciprocal(rc[:], dn[:])
                          ob = out1[:, t * D1:(t + 1) * D1]
                          nc.vector.tensor_tensor(
                              out=ob.rearrange("p (h f) -> p h f", h=HEADS),
                              in0=acc[:, 0:D1].rearrange("p (h f) -> p h f", h=HEADS),
                              in1=rc[:].unsqueeze(2).to_broadcast([P, HEADS, HID]),
                              op=A.mult)

              with tc.tile_pool(name="p1e", bufs=1) as pe:
                  b1_sb = pe.tile([P, D1], F32)
                  nc.sync.dma_start(b1_sb[:], b1_d[:])
                  if "epi" in SKIP and "ph1" not in SKIP:
                      nc.vector.memset(out1[:], 0.0)
                  for t in range(NT if ("ph1" not in SKIP and "epi" not in SKIP) else 0):
                      ob = out1[:, t * D1:(t + 1) * D1]
                      nc.vector.tensor_tensor(out=ob, in0=ob, in1=b1_sb[:],
                                              op=A.add)
                      nc.vector.tensor_scalar(out=ob, in0=ob, scalar1=0.0,
                                              scalar2=None, op0=A.max)

              # ---------------- Phase 2: h2 = relu(out1) @ W2, table2 -------------
              with tc.tile_pool(name="ph2", bufs=1) as sp, \
                   tc.tile_pool(name="ph2b", bufs=2) as sp2, \
                   tc.tile_pool(name="ph2p", bufs=2, space="PSUM") as pp:
                  w2_sb = sp.tile([P, 2, HID], F32)
                  nc.sync.dma_start(w2_sb[:],
                                    W2_d[:].rearrange("(k p) n -> p k n", p=P))
                  as2_sb = sp.tile([P, HID], F32)
                  nc.sync.dma_start(as2_sb[:], as2_d[:])
                  ad2_sb = sp.tile([P, HID], F32)
                  nc.sync.dma_start(ad2_sb[:], ad2_d[:])
                  for t in range(NT if "ph2" not in SKIP else 0):
                      h2_ps = pp.tile([P, HID], F32, space="PSUM")
                      for k in range(2):
                          hT_ps = pp.tile([P, P], F32, space="PSUM")
                          nc.tensor.transpose(
                              hT_ps[:],
                              out1[:, t * D1 + k * P:t * D1 + (k + 1) * P],
                              ident[:])
                          hT = sp2.tile([P, P], F32)
                          nc.vector.tensor_copy(out=hT[:], in_=hT_ps[:])
                          nc.tensor.matmul(h2_ps[:], lhsT=hT[:],
                                           rhs=w2_sb[:, k, :],
                                           start=(k == 0), stop=(k == 1))
                      t2t = sp2.tile([P, ROW2], F32)
                      tmp = sp2.tile([P, HID], F32)
                      nc.vector.tensor_tensor(out=tmp[:], in0=h2_ps[:],
                                              in1=as2_sb[:], op=A.mult)
                      nc.vector.tensor_reduce(out=t2t[:, OFF2:OFF2 + 1],
                                              in_=tmp[:], axis=X, op=A.add)
                      nc.vector.tensor_tensor(out=tmp[:], in0=h2_ps[:],
                                              in1=ad2_sb[:], op=A.mult)
                      nc.vector.tensor_reduce(out=t2t[:, OFF2 + 1:OFF2 + 2],
                                              in_=tmp[:], axis=X, op=A.add)
                      nc.vector.tensor_copy(out=t2t[:, 0:HID], in_=h2_ps[:])
                      nc.vector.memset(t2t[:, OFF2 + 2:ROW2], 0.0)
                      nc.sync.dma_start(t2loc[t * P:(t + 1) * P, :], t2t[:])
                  if "ag" not in SKIP:
                      nc.gpsimd.collective_compute(
                          "AllGather", A.bypass, replica_groups=rg,
                          ins=[t2loc[:]], outs=[t2full[:]])

            # ---------------- Phase 3: layer-2 edge phase + pooling -------------
            with tc.tile_pool(name="p3g", bufs=2) as pg, \
                 tc.tile_pool(name="p3gd", bufs=2) as pgd, \
                 tc.tile_pool(name="p3i", bufs=3) as pi, \
                 tc.tile_pool(name="p3w", bufs=2) as pw, \
                 tc.tile_pool(name="p3oh", bufs=2) as poh, \
                 tc.tile_pool(name="p3c", bufs=1) as pc, \
                 tc.tile_pool(name="p3ps", bufs=2, space="PSUM") as pps, \
                 tc.tile_pool(name="p3pl", bufs=1, space="PSUM") as ppl:
                b2_sb = pc.tile([P, HID], F32)
                nc.sync.dma_start(b2_sb[:], b2_d[:])
                pool_ps = ppl.tile([GPC, HID + 1], F32, space="PSUM")
                if "ph3" in SKIP or "epi" in SKIP:
                    nc.vector.memset(pool_ps[:], 1.0)
                for t in range(NT if "ph3" not in SKIP else 0):
                    acc = pps.tile([P, HID + 1], F32, space="PSUM", bufs=3)
                    for hh in range(2):
                        gbase = (t * 2 + hh) * NIc
                        cbase = (t * 2 + hh) * CH
                        G = pg.tile([P, CH, ROW2], F32)
                        if "gsrc" not in SKIP:
                            nc.gpsimd.dma_gather(
                                G[:], t2full[:],
                                ixs_all[:, gbase:gbase + NIc],
                                NI, NI, ROW2, single_packet=SP)
                        Gd = pgd.tile([P, CH, 64], F32)
                        asr = G[:, :, OFF2:OFF2 + 1]
                        if "gdst" not in SKIP:
                            nc.gpsimd.dma_gather(
                                Gd[:], t2full[:, OFF2:OFF2 + 64],
                                ixd_all[:, gbase:gbase + NIc],
                                NI, NI, 64, elem_step=ROW2, single_packet=SP)
                            nc.vector.tensor_tensor(out=asr, in0=asr,
                                                    in1=Gd[:, :, 1:2],
                                                    op=A.add)
                        if "msg" not in SKIP:
                            ae = pw.tile([P, CH], F32)
                            nc.vector.tensor_scalar(
                                out=ae[:], in0=wv_sb[:, cbase:cbase + CH],
                                scalar1=q2_sb[:, 0:1], scalar2=None, op0=A.mult)
                            nc.vector.tensor_tensor(out=asr, in0=asr,
                                                        in1=ae[:].unsqueeze(2),
                                                        op=A.add)
                            e2 = pw.tile([P, CH, 1], F32)
                            nc.scalar.activation(out=e2[:], in_=asr, func=ACT.Exp,
                                                 scale=0.2)
                            nc.scalar.activation(out=asr, in_=asr, func=ACT.Exp)
                            nc.vector.tensor_tensor(out=asr, in0=asr, in1=e2[:],
                                                        op=A.max)
                            gm = G[:, :, 0:HID]
                            p_b = asr.to_broadcast([P, CH, HID])
                            nc.vector.tensor_tensor(out=gm, in0=gm, in1=p_b,
                                                        op=A.mult)
                        if "mm" not in SKIP:
                            oh = poh.tile([P, CH, P], F32)
                            nc.vector.tensor_tensor(
                                out=oh[:],
                                in0=iota_sb[:].unsqueeze(1)
                                    .to_broadcast([P, CH, P]),
                                in1=dstl_sb[:, cbase:cbase + CH]
                                    .unsqueeze(2).to_broadcast([P, CH, P]),
                                op=A.is_equal)
                            for c in range(CH):
                                nc.tensor.matmul(
                                    acc[:], lhsT=oh[:, c, :],
                                    rhs=G[:, c, 0:HID + 1],
                                    start=(hh == 0 and c == 0),
                                    stop=(hh == 1 and c == CH - 1))
                    if "epi" not in SKIP:
                        # epilogue: out2e = [relu(acc/denom + b2) | 1]
                        dn = pw.tile([P, 1], F32)
                        nc.vector.tensor_scalar(out=dn[:], in0=acc[:, HID:HID + 1],
                                                scalar1=1e-16, scalar2=None,
                                                op0=A.add)
                        rc = pw.tile([P, 1], F32)
                        nc.vector.reciprocal(rc[:], dn[:])
                        o2 = pw.tile([P, HID + 1], F32)
                        nc.vector.tensor_scalar(out=o2[:, 0:HID], in0=acc[:, 0:HID],
                                                scalar1=rc[:, 0:1], scalar2=None,
                                                op0=A.mult)
                        nc.vector.tensor_tensor(out=o2[:, 0:HID], in0=o2[:, 0:HID],
                                                in1=b2_sb[:], op=A.add)
                        nc.vector.tensor_scalar(out=o2[:, 0:HID], in0=o2[:, 0:HID],
                                                scalar1=0.0, scalar2=None, op0=A.max)
                        nc.vector.memset(o2[:, HID:HID + 1], 1.0)
                        ohg = poh.tile([P, GPC], F32)
                        nc.vector.tensor_scalar(
                            out=ohg[:], in0=iota_sb[:, 0:GPC],
                            scalar1=gl_sb[:, t:t + 1], scalar2=None, op0=A.is_equal)
                        nc.tensor.matmul(pool_ps[:], lhsT=ohg[:], rhs=o2[:],
                                         start=(t == 0), stop=(t == NT - 1),
                                         skip_group_check=True)

                # ------------- Phase 4: pooled mean + FC ------------------------
                fcw_sb = pc.tile([HID, OUT], F32)
                nc.sync.dma_start(fcw_sb[:], fcw_d[:])
                fcb_sb = pc.tile([P, OUT], F32)
                nc.sync.dma_start(fcb_sb[:], fcb_d[:])
                cnt = pc.tile([GPC, 1], F32)
                nc.vector.tensor_scalar(out=cnt[:], in0=pool_ps[:, HID:HID + 1],
                                        scalar1=1.0, scalar2=None, op0=A.max)
                rcc = pc.tile([GPC, 1], F32)
                nc.vector.reciprocal(rcc[:], cnt[:])
                pooled = pc.tile([GPC, HID], F32)
                nc.vector.tensor_scalar(out=pooled[:], in0=pool_ps[:, 0:HID],
                                        scalar1=rcc[:, 0:1], scalar2=None,
                                        op0=A.mult)
                pT_ps = pps.tile([HID, GPC], F32, space="PSUM")
                nc.tensor.transpose(pT_ps[:], pooled[:], ident[:GPC, :GPC])
                pT = pc.tile([HID, GPC], F32)
                nc.vector.tensor_copy(out=pT[:], in_=pT_ps[:])
                fc_ps = pps.tile([GPC, OUT], F32, space="PSUM")
                nc.tensor.matmul(fc_ps[:], lhsT=pT[:], rhs=fcw_sb[:],
                                 start=True, stop=True)
                res = pc.tile([GPC, OUT], F32)
                nc.vector.tensor_tensor(out=res[:], in0=fc_ps[:],
                                        in1=fcb_sb[:GPC, :], op=A.add)
                nc.sync.dma_start(out_d[:], res[:])

    nc.compile()
    return nc


# ---------------------------------------------------------------------------
# Entry point.
# ---------------------------------------------------------------------------
def run(inputs, cfg, **run_kwargs):
    in_maps, meta = prepare(inputs, cfg)
    nc = build(meta)
    res = run_bass_kernel_spmd(nc, in_maps, core_ids=list(range(NCORES)),
                               **run_kwargs)
    out = np.concatenate([res.results[c]["out"] for c in range(NCORES)],
                         axis=0)
    return np.asarray(out, np.float32), res


def kernel(**inputs) -> np.ndarray:
    out, _ = run(inputs, FULL_CFG)
    return out



# revision 5
# speedup vs baseline: 1.7053x; 1.7053x over previous
"""Trainium2 Bass kernel for a 2-layer edge-featured GAT + mean-pool + FC.

Sharding: 256 graphs split 32-per-core across 8 cores; batch is sorted so each
core owns a contiguous, graph-aligned node range. Edges live on the core that
owns their destination node. Per layer, each core's node table (features +
attention terms) is AllGathered so any core can gather arbitrary source rows.

Node-table rows are bf16: layer-1 rows are 384 bf16 elems (768B) laid out as
[h(256 bf16) | a_src(4 fp32, bitcast) | pad]; layer-2 rows are 128 bf16 elems
(256B) as [h2(64) | a_src2(1 fp32 bitcast) | pad]. Only ONE dma_gather per
128-node destination tile pulls all its (padded) edges' source rows; the
destination-side attention term a_dst needs no DRAM gather at all: since every
edge's destination is one of the tile's 128 local nodes, a transposed one-hot
ohT[node, edge] (built from a uint8 replicated dst-local index row with a
single is_equal) turns the lookup into tiny per-chunk matmuls
ae[e,h] = sum_n ohT[n,e] * a_dst[n,h] on the tensor engine.

Attention weights p = exp(leaky_relu(a_src+a_dst+w*q)) are computed as
max(exp(x), exp(0.2x)), written as bf16 into the gathered rows' pad columns,
messages are h*p, and the per-destination segment-sum is a one-hot matmul
accumulated in PSUM, with p itself carried as extra columns to produce the
softmax denominators. Normalization happens once per node after aggregation
(the reference's max-subtraction cancels exactly in the ratio). Pad edge
slots gather row 0 (finite) and carry dst_local = -1 so their one-hot columns
are all zero and they contribute nothing.
"""

import sys

sys.path.insert(0, "/opt/trn_rl_repo")

import math
from contextlib import ExitStack

import ml_dtypes
import numpy as np

import concourse.bacc as bacc
import concourse.bass as bass
import concourse.mybir as mybir
import concourse.tile as tile
from concourse.bass_utils import run_bass_kernel_spmd
from concourse.masks import make_identity

P = 128
NCORES = 8

FULL_CFG = dict(N=20000, E=640000, FIN=128, HID=64, HEADS=4, NG=256, OUT=32)

F32 = mybir.dt.float32
BF16 = mybir.dt.bfloat16
U8 = mybir.dt.uint8
I16 = mybir.dt.int16

BF = ml_dtypes.bfloat16

ROW1 = 384   # bf16 elems: h(256) | a_src fp32 (8 slots) | pad -> 768B
ROW2 = 128   # bf16 elems: h2(64) | a_src2 fp32 (2 slots) | pad -> 256B


# ---------------------------------------------------------------------------
# Host-side preparation: integer index manipulation + array reordering only.
# ---------------------------------------------------------------------------
def prepare(inputs, cfg):
    N, E, FIN, HID, HEADS, NG, OUT = (
        cfg["N"], cfg["E"], cfg["FIN"], cfg["HID"], cfg["HEADS"], cfg["NG"],
        cfg["OUT"],
    )
    GPC = NG // NCORES  # graphs per core

    x = np.asarray(inputs["x"], np.float32)
    ei = np.asarray(inputs["edge_index"], np.int64)
    ea = np.asarray(inputs["edge_attr"], np.float32)
    batch = np.asarray(inputs["batch"], np.int64)
    src, dst = ei[0], ei[1]

    # node ranges per core (graph-aligned; batch is sorted)
    bounds = np.searchsorted(batch, np.arange(NCORES + 1) * GPC)
    node_cnt = np.diff(bounds)
    NT = max(1, math.ceil(node_cnt.max() / P))
    NSLICE = NT * P
    NROWS = NCORES * NSLICE
    assert NROWS < 32768, f"int16 gather index overflow: {NROWS}"

    core_of_node = np.minimum(batch // GPC, NCORES - 1).astype(np.int64)
    rowid = np.empty(N, np.int64)
    for c in range(NCORES):
        ns, ne = bounds[c], bounds[c + 1]
        rowid[ns:ne] = c * NSLICE + np.arange(ne - ns)

    # edges sorted by dst; since batch is sorted, core blocks are contiguous
    order = np.argsort(dst, kind="stable")
    dsts = dst[order]
    srcs = src[order]
    ws = ea[order, 0]
    ecore = core_of_node[dsts]
    ebounds = np.searchsorted(ecore, np.arange(NCORES + 1))

    # chunks-per-tile: max over all (core, tile)
    cpt_max = 1
    tile_edge_counts = []
    for c in range(NCORES):
        es, ee = ebounds[c], ebounds[c + 1]
        dln = dsts[es:ee] - bounds[c]
        tid = dln // P
        cnts = np.bincount(tid, minlength=NT)
        tile_edge_counts.append(cnts)
        if len(cnts):
            cpt_max = max(cpt_max, math.ceil(cnts.max() / P))
    CPT = cpt_max
    NCHUNK = NT * CPT

    per_core = []
    for c in range(NCORES):
        ns, ne = bounds[c], bounds[c + 1]
        es, ee = ebounds[c], ebounds[c + 1]
        nloc = ne - ns

        xs = np.zeros((NSLICE, FIN), np.float32)
        xs[:nloc] = x[ns:ne]

        gl = np.full((NT * P,), -1.0, np.float32)
        gl[:nloc] = (batch[ns:ne] - c * GPC).astype(np.float32)
        gl_dev = gl.reshape(NT, P).T.copy()  # [128, NT]

        srcrow = np.zeros((NT, CPT * P), np.int64)   # pad -> row 0 (finite)
        dstl = np.full((NT, CPT * P), -1.0, np.float32)
        dstu8 = np.full((NT, CPT * P), 255, np.uint8)
        wv = np.zeros((NT, CPT * P), np.float32)

        dln = dsts[es:ee] - ns
        cnts = tile_edge_counts[c]
        off = np.zeros(NT + 1, np.int64)
        off[1:NT + 1] = np.cumsum(cnts[:NT])
        for t in range(NT):
            k = int(cnts[t]) if t < len(cnts) else 0
            if k == 0:
                continue
            sel = slice(es + int(off[t]), es + int(off[t]) + k)
            srcrow[t, :k] = rowid[srcs[sel]]
            loc = (dln[int(off[t]):int(off[t]) + k] % P)
            dstl[t, :k] = loc.astype(np.float32)
            dstu8[t, :k] = loc.astype(np.uint8)
            wv[t, :k] = ws[sel]

        # device layouts
        dstl_dev = dstl.reshape(NCHUNK, P).T.astype(BF).copy()  # [128, NCHUNK]
        wv_dev = wv.reshape(NCHUNK, P).T.copy()

        def wrap_idx(arr):  # [NT, CPT*P] -> [128, NT*CPT*8] int16
            blocks = []
            for t in range(NT):
                a = arr[t].reshape(CPT * 8, 16).T  # [16, CPT*8]
                blocks.append(np.tile(a, (8, 1)))
            return np.ascontiguousarray(
                np.concatenate(blocks, axis=1)).astype(np.int16)

        per_core.append(dict(
            xs=xs.astype(BF), gl=gl_dev, dstl=dstl_dev, wv=wv_dev,
            du=dstu8[None],                  # [1, NT, CPT*P] uint8
            ixs=wrap_idx(srcrow),
        ))

    # weight-side constants (tiny, host-replicated)
    W1 = np.asarray(inputs["W1"], np.float32)            # [FIN, H*HID]
    W2 = np.asarray(inputs["W2"], np.float32)            # [H*HID, HID]
    as1 = np.asarray(inputs["att_src1"], np.float32).reshape(-1)
    ad1 = np.asarray(inputs["att_dst1"], np.float32).reshape(-1)
    as2 = np.asarray(inputs["att_src2"], np.float32).reshape(-1)
    ad2 = np.asarray(inputs["att_dst2"], np.float32).reshape(-1)
    q1 = (np.asarray(inputs["We1"], np.float32).reshape(HEADS, HID)
          * np.asarray(inputs["att_edge1"], np.float32)).sum(axis=1)  # [H]
    q2 = float((np.asarray(inputs["We2"], np.float32).reshape(-1)
                * np.asarray(inputs["att_edge2"], np.float32).reshape(-1))
               .sum())
    b1 = np.asarray(inputs["b1"], np.float32)
    b2 = np.asarray(inputs["b2"], np.float32)
    fcW = np.asarray(inputs["fcW"], np.float32)
    fcb = np.asarray(inputs["fcb"], np.float32)

    rep = lambda vv: np.tile(vv[None, :].astype(np.float32), (P, 1)).copy()
    consts = dict(
        W1=W1.astype(BF),
        W2=np.ascontiguousarray(
            W2.reshape(2, P, HID).transpose(1, 0, 2)).astype(BF),  # [P,2,HID]
        as1b=rep(as1), ad1b=rep(ad1), b1b=rep(b1),
        as2b=rep(as2), ad2b=rep(ad2), b2b=rep(b2),
        q1b=rep(q1), q2b=np.full((P, 1), q2, np.float32),
        fcw=fcW, fcbb=rep(fcb),
        iota=np.tile(np.arange(P, dtype=np.float32)[None, :], (P, 1)).copy(),
        iotab=np.tile(np.arange(P, dtype=np.float32)[None, :],
                      (P, 1)).astype(BF),
        iotac=np.arange(P, dtype=np.float32)[:, None].copy(),
    )

    in_maps = []
    for c in range(NCORES):
        m = dict(per_core[c])
        m.update(consts)
        in_maps.append(m)

    meta = dict(NT=NT, CPT=CPT, NSLICE=NSLICE, NROWS=NROWS, GPC=GPC, **cfg)
    return in_maps, meta


# ---------------------------------------------------------------------------
# Device program.
# ---------------------------------------------------------------------------
def build(meta, num_devices=NCORES):
    NT, CPT = meta["NT"], meta["CPT"]
    NSLICE, NROWS, GPC = meta["NSLICE"], meta["NROWS"], meta["GPC"]
    FIN, HID, HEADS, OUT = meta["FIN"], meta["HID"], meta["HEADS"], meta["OUT"]
    D1 = HEADS * HID          # 256
    NI = CPT * P              # gather idxs per tile
    NIc = NI // 16            # idx columns per tile (= CPT*8)
    NCHUNK = NT * CPT
    A = mybir.AluOpType
    ACT = mybir.ActivationFunctionType
    X = mybir.AxisListType.X
    rg = [list(range(NCORES))]

    nc = bacc.Bacc("TRN2", target_bir_lowering=False, debug=False,
                   num_devices=num_devices,
                   dynamic_dma_scratch_size=65536)

    def din(name, shape, dtype=F32):
        return nc.dram_tensor(name, list(shape), dtype,
                              kind="ExternalInput").ap()

    xs = din("xs", (NSLICE, FIN), BF16)
    ixs_d = din("ixs", (P, NT * NIc), I16)
    du_d = din("du", (1, NT, NI), U8)
    dstl_d = din("dstl", (P, NCHUNK), BF16)
    wv_d = din("wv", (P, NCHUNK))
    gl_d = din("gl", (P, NT))
    W1_d = din("W1", (FIN, D1), BF16)
    W2_d = din("W2", (P, 2, HID), BF16)
    as1_d = din("as1b", (P, D1))
    ad1_d = din("ad1b", (P, D1))
    b1_d = din("b1b", (P, D1))
    as2_d = din("as2b", (P, HID))
    ad2_d = din("ad2b", (P, HID))
    b2_d = din("b2b", (P, HID))
    q1_d = din("q1b", (P, HEADS))
    q2_d = din("q2b", (P, 1))
    fcw_d = din("fcw", (HID, OUT))
    fcb_d = din("fcbb", (P, OUT))
    iota_d = din("iota", (P, P))
    iotab_d = din("iotab", (P, P), BF16)
    iotac_d = din("iotac", (P, 1))

    out_d = nc.dram_tensor("out", [GPC, OUT], F32, kind="ExternalOutput").ap()

    with tile.TileContext(nc) as tc, ExitStack() as st:
        constp = st.enter_context(tc.tile_pool(name="constp", bufs=1))
        drp = st.enter_context(tc.tile_pool(name="drp", bufs=1, space="DRAM"))

        # whole-kernel constants
        iota_sb = constp.tile([P, P], F32)
        nc.sync.dma_start(iota_sb[:], iota_d[:])
        iotab_sb = constp.tile([P, P], BF16)
        nc.sync.dma_start(iotab_sb[:], iotab_d[:])
        iotac_sb = constp.tile([P, 1], F32)
        nc.sync.dma_start(iotac_sb[:], iotac_d[:])
        identf = constp.tile([P, P], F32)
        make_identity(nc, identf[:])
        identb = constp.tile([P, P], BF16)
        make_identity(nc, identb[:])
        dstl_sb = constp.tile([P, NCHUNK], BF16)
        nc.sync.dma_start(dstl_sb[:], dstl_d[:])
        wv_sb = constp.tile([P, NCHUNK], F32)
        nc.sync.dma_start(wv_sb[:], wv_d[:])
        gl_sb = constp.tile([P, NT], F32)
        nc.sync.dma_start(gl_sb[:], gl_d[:])
        q1_sb = constp.tile([P, HEADS], F32)
        nc.sync.dma_start(q1_sb[:], q1_d[:])
        q2_sb = constp.tile([P, 1], F32)
        nc.sync.dma_start(q2_sb[:], q2_d[:])
        ixs_all = constp.tile([P, NT * NIc], I16)
        nc.sync.dma_start(ixs_all[:], ixs_d[:])
        adst_all = constp.tile([P, NT, HEADS], BF16)
        adst2_all = constp.tile([P, NT], BF16)
        out1 = constp.tile([P, NT * D1], F32)

        t1loc = drp.tile([NSLICE, ROW1], BF16, name="t1loc")
        t1full = drp.tile([NROWS, ROW1], BF16, addr_space="Shared",
                          name="t1full")
        t2loc = drp.tile([NSLICE, ROW2], BF16, name="t2loc")
        t2full = drp.tile([NROWS, ROW2], BF16, addr_space="Shared",
                          name="t2full")

        # ---------------- Phase 0: h1 = x @ W1, a_src/a_dst, table1 ---------
        with tc.tile_pool(name="ph0", bufs=1) as sp, \
             tc.tile_pool(name="ph0b", bufs=2) as sp2, \
             tc.tile_pool(name="ph0p", bufs=2, space="PSUM") as pp:
            w1_sb = sp.tile([P, D1], BF16)
            nc.sync.dma_start(w1_sb[:], W1_d[:])
            as1_sb = sp.tile([P, D1], F32)
            nc.sync.dma_start(as1_sb[:], as1_d[:])
            ad1_sb = sp.tile([P, D1], F32)
            nc.sync.dma_start(ad1_sb[:], ad1_d[:])
            xall = sp.tile([P, NT, FIN], BF16)
            nc.sync.dma_start(xall[:],
                              xs[:].rearrange("(t p) f -> p t f", p=P))
            for t in range(NT):
                xT_ps = pp.tile([P, P], BF16, space="PSUM")
                nc.tensor.transpose(xT_ps[:], xall[:, t, :], identb[:])
                xT = sp2.tile([P, P], BF16)
                nc.vector.tensor_copy(out=xT[:], in_=xT_ps[:])
                h_ps = pp.tile([P, D1], F32, space="PSUM")
                nc.tensor.matmul(h_ps[:], lhsT=xT[:], rhs=w1_sb[:],
                                 start=True, stop=True)
                t1t = sp2.tile([P, ROW1], BF16)
                tmp = sp2.tile([P, D1], F32)
                nc.vector.tensor_tensor(out=tmp[:], in0=h_ps[:],
                                        in1=as1_sb[:], op=A.mult)
                nc.vector.tensor_reduce(
                    out=t1t[:, D1:D1 + 2 * HEADS].bitcast(F32),
                    in_=tmp[:].rearrange("p (h f) -> p h f", h=HEADS),
                    axis=X, op=A.add)
                nc.vector.tensor_tensor(out=tmp[:], in0=h_ps[:],
                                        in1=ad1_sb[:], op=A.mult)
                adf = sp2.tile([P, HEADS], F32)
                nc.vector.tensor_reduce(
                    out=adf[:],
                    in_=tmp[:].rearrange("p (h f) -> p h f", h=HEADS),
                    axis=X, op=A.add)
                nc.vector.tensor_copy(out=adst_all[:, t, :], in_=adf[:])
                nc.vector.tensor_copy(out=t1t[:, 0:D1], in_=h_ps[:])
                nc.vector.memset(t1t[:, D1 + 2 * HEADS:ROW1], 0.0)
                nc.sync.dma_start(t1loc[t * P:(t + 1) * P, :], t1t[:])
            nc.gpsimd.collective_compute(
                "AllGather", A.bypass, replica_groups=rg,
                ins=[t1loc[:]], outs=[t1full[:]])

        # ---------------- Phase 1: layer-1 edge phase -----------------------
        with tc.tile_pool(name="p1g", bufs=2) as pg, \
             tc.tile_pool(name="p1u", bufs=2) as pu, \
             tc.tile_pool(name="p1o", bufs=2) as po, \
             tc.tile_pool(name="p1w", bufs=3) as pw, \
             tc.tile_pool(name="p1e", bufs=1) as pe, \
             tc.tile_pool(name="p1ps", bufs=2, space="PSUM") as pps, \
             tc.tile_pool(name="p1pa", bufs=2, space="PSUM") as ppa:
            b1_sb = pe.tile([P, D1], F32)
            nc.sync.dma_start(b1_sb[:], b1_d[:])
            for t in range(NT):
                G = pg.tile([P, CPT, ROW1], BF16)
                nc.gpsimd.dma_gather(
                    G[:], t1full[:], ixs_all[:, t * NIc:(t + 1) * NIc],
                    NI, NI, ROW1, single_packet=False)
                du = pu.tile([P, NI], U8)
                nc.sync.dma_start(du[:],
                                  du_d[0:1, t, :].partition_broadcast(P))
                ohT = po.tile([P, NI], BF16)
                nc.vector.tensor_scalar(out=ohT[:], in0=du[:],
                                        scalar1=iotac_sb[:, 0:1],
                                        scalar2=None, op0=A.is_equal)
                ae_ps = ppa.tile([P, CPT, HEADS], F32, space="PSUM")
                for c in range(CPT):
                    nc.tensor.matmul(ae_ps[:, c, :],
                                     lhsT=ohT[:, c * P:(c + 1) * P],
                                     rhs=adst_all[:, t, :],
                                     start=True, stop=True)
                oh = po.tile([P, CPT, P], BF16)
                nc.vector.tensor_tensor(
                    out=oh[:],
                    in0=iotab_sb[:].unsqueeze(1).to_broadcast([P, CPT, P]),
                    in1=dstl_sb[:, t * CPT:(t + 1) * CPT]
                        .unsqueeze(2).to_broadcast([P, CPT, P]),
                    op=A.is_equal)
                # alpha = a_src + a_dst + w*q ; p = max(exp(a), exp(0.2a))
                asr = pw.tile([P, CPT, HEADS], F32)
                nc.vector.tensor_tensor(
                    out=asr[:],
                    in0=wv_sb[:, t * CPT:(t + 1) * CPT].unsqueeze(2)
                        .to_broadcast([P, CPT, HEADS]),
                    in1=q1_sb[:].unsqueeze(1).to_broadcast([P, CPT, HEADS]),
                    op=A.mult)
                nc.vector.tensor_tensor(
                    out=asr[:], in0=asr[:],
                    in1=G[:, :, D1:D1 + 2 * HEADS].bitcast(F32), op=A.add)
                nc.vector.tensor_tensor(out=asr[:], in0=asr[:], in1=ae_ps[:],
                                        op=A.add)
                e2 = pw.tile([P, CPT, HEADS], F32)
                nc.scalar.activation(out=e2[:], in_=asr[:], func=ACT.Exp,
                                     scale=0.2)
                nc.scalar.activation(out=asr[:], in_=asr[:], func=ACT.Exp)
                pbf = G[:, :, D1:D1 + HEADS]   # bf16 p slot
                nc.vector.tensor_tensor(out=pbf, in0=asr[:], in1=e2[:],
                                        op=A.max)
                gm = G[:, :, 0:D1].rearrange("p c (h f) -> p c h f", h=HEADS)
                nc.vector.tensor_tensor(
                    out=gm, in0=gm,
                    in1=pbf.unsqueeze(3).to_broadcast([P, CPT, HEADS, HID]),
                    op=A.mult)
                acc = pps.tile([P, D1 + HEADS], F32, space="PSUM")
                for c in range(CPT):
                    nc.tensor.matmul(acc[:], lhsT=oh[:, c, :],
                                     rhs=G[:, c, 0:D1 + HEADS],
                                     start=(c == 0), stop=(c == CPT - 1))
                # epilogue: out1 = relu(acc/denom + b1)
                dn = pw.tile([P, HEADS], F32)
                nc.vector.tensor_scalar(out=dn[:], in0=acc[:, D1:D1 + HEADS],
                                        scalar1=1e-16, scalar2=None,
                                        op0=A.add)
                rc = pw.tile([P, HEADS], F32)
                nc.vector.reciprocal(rc[:], dn[:])
                ob = out1[:, t * D1:(t + 1) * D1]
                nc.vector.tensor_tensor(
                    out=ob.rearrange("p (h f) -> p h f", h=HEADS),
                    in0=acc[:, 0:D1].rearrange("p (h f) -> p h f", h=HEADS),
                    in1=rc[:].unsqueeze(2).to_broadcast([P, HEADS, HID]),
                    op=A.mult)
                nc.vector.tensor_tensor(out=ob, in0=ob, in1=b1_sb[:],
                                        op=A.add)
                nc.vector.tensor_scalar(out=ob, in0=ob, scalar1=0.0,
                                        scalar2=None, op0=A.max)

        # ---------------- Phase 2: h2 = out1 @ W2, table2 -------------------
        with tc.tile_pool(name="ph2", bufs=1) as sp, \
             tc.tile_pool(name="ph2b", bufs=2) as sp2, \
             tc.tile_pool(name="ph2p", bufs=2, space="PSUM") as pp:
            w2_sb = sp.tile([P, 2, HID], BF16)
            nc.sync.dma_start(w2_sb[:], W2_d[:])
            as2_sb = sp.tile([P, HID], F32)
            nc.sync.dma_start(as2_sb[:], as2_d[:])
            ad2_sb = sp.tile([P, HID], F32)
            nc.sync.dma_start(ad2_sb[:], ad2_d[:])
            for t in range(NT):
                h2_ps = pp.tile([P, HID], F32, space="PSUM")
                for k in range(2):
                    hT_ps = pp.tile([P, P], F32, space="PSUM")
                    nc.tensor.transpose(
                        hT_ps[:],
                        out1[:, t * D1 + k * P:t * D1 + (k + 1) * P],
                        identf[:])
                    hT = sp2.tile([P, P], BF16)
                    nc.vector.tensor_copy(out=hT[:], in_=hT_ps[:])
                    nc.tensor.matmul(h2_ps[:], lhsT=hT[:],
                                     rhs=w2_sb[:, k, :],
                                     start=(k == 0), stop=(k == 1))
                t2t = sp2.tile([P, ROW2], BF16)
                tmp = sp2.tile([P, HID], F32)
                nc.vector.tensor_tensor(out=tmp[:], in0=h2_ps[:],
                                        in1=as2_sb[:], op=A.mult)
                nc.vector.tensor_reduce(out=t2t[:, HID:HID + 2].bitcast(F32),
                                        in_=tmp[:], axis=X, op=A.add)
                nc.vector.tensor_tensor(out=tmp[:], in0=h2_ps[:],
                                        in1=ad2_sb[:], op=A.mult)
                ad2f = sp2.tile([P, 1], F32)
                nc.vector.tensor_reduce(out=ad2f[:], in_=tmp[:], axis=X,
                                        op=A.add)
                nc.vector.tensor_copy(out=adst2_all[:, t:t + 1], in_=ad2f[:])
                nc.vector.tensor_copy(out=t2t[:, 0:HID], in_=h2_ps[:])
                nc.vector.memset(t2t[:, HID + 2:ROW2], 0.0)
                nc.sync.dma_start(t2loc[t * P:(t + 1) * P, :], t2t[:])
            nc.gpsimd.collective_compute(
                "AllGather", A.bypass, replica_groups=rg,
                ins=[t2loc[:]], outs=[t2full[:]])

        # ---------------- Phase 3: layer-2 edge phase + pooling -------------
        with tc.tile_pool(name="p3g", bufs=2) as pg, \
             tc.tile_pool(name="p3u", bufs=2) as pu, \
             tc.tile_pool(name="p3o", bufs=2) as po, \
             tc.tile_pool(name="p3w", bufs=3) as pw, \
             tc.tile_pool(name="p3c", bufs=1) as pc, \
             tc.tile_pool(name="p3ps", bufs=2, space="PSUM") as pps, \
             tc.tile_pool(name="p3pa", bufs=2, space="PSUM") as ppa, \
             tc.tile_pool(name="p3pl", bufs=1, space="PSUM") as ppl:
            b2_sb = pc.tile([P, HID], F32)
            nc.sync.dma_start(b2_sb[:], b2_d[:])
            pool_ps = ppl.tile([GPC, HID + 1], F32, space="PSUM")
            for t in range(NT):
                G = pg.tile([P, CPT, ROW2], BF16)
                nc.gpsimd.dma_gather(
                    G[:], t2full[:], ixs_all[:, t * NIc:(t + 1) * NIc],
                    NI, NI, ROW2, single_packet=False)
                du = pu.tile([P, NI], U8)
                nc.sync.dma_start(du[:],
                                  du_d[0:1, t, :].partition_broadcast(P))
                ohT = po.tile([P, NI], BF16)
                nc.vector.tensor_scalar(out=ohT[:], in0=du[:],
                                        scalar1=iotac_sb[:, 0:1],
                                        scalar2=None, op0=A.is_equal)
                ae_ps = ppa.tile([P, CPT, 1], F32, space="PSUM")
                for c in range(CPT):
                    nc.tensor.matmul(ae_ps[:, c, :],
                                     lhsT=ohT[:, c * P:(c + 1) * P],
                                     rhs=adst2_all[:, t:t + 1],
                                     start=True, stop=True)
                oh = po.tile([P, CPT, P], BF16)
                nc.vector.tensor_tensor(
                    out=oh[:],
                    in0=iotab_sb[:].unsqueeze(1).to_broadcast([P, CPT, P]),
                    in1=dstl_sb[:, t * CPT:(t + 1) * CPT]
                        .unsqueeze(2).to_broadcast([P, CPT, P]),
                    op=A.is_equal)
                asr = pw.tile([P, CPT, 1], F32)
                nc.vector.tensor_scalar(
                    out=asr[:], in0=wv_sb[:, t * CPT:(t + 1) * CPT]
                        .unsqueeze(2),
                    scalar1=q2_sb[:, 0:1], scalar2=None, op0=A.mult)
                nc.vector.tensor_tensor(
                    out=asr[:], in0=asr[:],
                    in1=G[:, :, HID:HID + 2].bitcast(F32), op=A.add)
                nc.vector.tensor_tensor(out=asr[:], in0=asr[:], in1=ae_ps[:],
                                        op=A.add)
                e2 = pw.tile([P, CPT, 1], F32)
                nc.scalar.activation(out=e2[:], in_=asr[:], func=ACT.Exp,
                                     scale=0.2)
                nc.scalar.activation(out=asr[:], in_=asr[:], func=ACT.Exp)
                pbf = G[:, :, HID:HID + 1]
                nc.vector.tensor_tensor(out=pbf, in0=asr[:], in1=e2[:],
                                        op=A.max)
                gm = G[:, :, 0:HID]
                nc.vector.tensor_tensor(
                    out=gm, in0=gm,
                    in1=pbf.to_broadcast([P, CPT, HID]), op=A.mult)
                acc = pps.tile([P, HID + 1], F32, space="PSUM")
                for c in range(CPT):
                    nc.tensor.matmul(acc[:], lhsT=oh[:, c, :],
                                     rhs=G[:, c, 0:HID + 1],
                                     start=(c == 0), stop=(c == CPT - 1))
                # epilogue: o2 = [relu(acc/denom + b2) | 1], pool matmul
                dn = pw.tile([P, 1], F32)
                nc.vector.tensor_scalar(out=dn[:], in0=acc[:, HID:HID + 1],
                                        scalar1=1e-16, scalar2=None,
                                        op0=A.add)
                rc = pw.tile([P, 1], F32)
                nc.vector.reciprocal(rc[:], dn[:])
                o2 = pw.tile([P, HID + 1], F32)
                nc.vector.tensor_scalar(out=o2[:, 0:HID], in0=acc[:, 0:HID],
                                        scalar1=rc[:, 0:1], scalar2=None,
                                        op0=A.mult)
                nc.vector.tensor_tensor(out=o2[:, 0:HID], in0=o2[:, 0:HID],
                                        in1=b2_sb[:], op=A.add)
                nc.vector.tensor_scalar(out=o2[:, 0:HID], in0=o2[:, 0:HID],
                                        scalar1=0.0, scalar2=None, op0=A.max)
                nc.vector.memset(o2[:, HID:HID + 1], 1.0)
                ohg = pw.tile([P, GPC], F32)
                nc.vector.tensor_scalar(
                    out=ohg[:], in0=iota_sb[:, 0:GPC],
                    scalar1=gl_sb[:, t:t + 1], scalar2=None, op0=A.is_equal)
                nc.tensor.matmul(pool_ps[:], lhsT=ohg[:], rhs=o2[:],
                                 start=(t == 0), stop=(t == NT - 1),
                                 skip_group_check=True)

            # ------------- Phase 4: pooled mean + FC ------------------------
            fcw_sb = pc.tile([HID, OUT], F32)
            nc.sync.dma_start(fcw_sb[:], fcw_d[:])
            fcb_sb = pc.tile([P, OUT], F32)
            nc.sync.dma_start(fcb_sb[:], fcb_d[:])
            cnt = pc.tile([GPC, 1], F32)
            nc.vector.tensor_scalar(out=cnt[:], in0=pool_ps[:, HID:HID + 1],
                                    scalar1=1.0, scalar2=None, op0=A.max)
            rcc = pc.tile([GPC, 1], F32)
            nc.vector.reciprocal(rcc[:], cnt[:])
            pooled = pc.tile([GPC, HID], F32)
            nc.vector.tensor_scalar(out=pooled[:], in0=pool_ps[:, 0:HID],
                                    scalar1=rcc[:, 0:1], scalar2=None,
                                    op0=A.mult)
            pT_ps = ppl.tile([HID, GPC], F32, space="PSUM")
            nc.tensor.transpose(pT_ps[:], pooled[:], identf[:GPC, :GPC])
            pT = pc.tile([HID, GPC], F32)
            nc.vector.tensor_copy(out=pT[:], in_=pT_ps[:])
            fc_ps = ppl.tile([GPC, OUT], F32, space="PSUM")
            nc.tensor.matmul(fc_ps[:], lhsT=pT[:], rhs=fcw_sb[:],
                             start=True, stop=True)
            res = pc.tile([GPC, OUT], F32)
            nc.vector.tensor_tensor(out=res[:], in0=fc_ps[:],
                                    in1=fcb_sb[:GPC, :], op=A.add)
            nc.sync.dma_start(out_d[:], res[:])

    nc.compile()
    return nc


# ---------------------------------------------------------------------------
# Entry point.
# ---------------------------------------------------------------------------
def run(inputs, cfg, **run_kwargs):
    in_maps, meta = prepare(inputs, cfg)
    nc = build(meta)
    res = run_bass_kernel_spmd(nc, in_maps, core_ids=list(range(NCORES)),
                               **run_kwargs)
    out = np.concatenate([res.results[c]["out"] for c in range(NCORES)],
                         axis=0)
    return np.asarray(out, np.float32), res


def kernel(**inputs) -> np.ndarray:
    out, _ = run(inputs, FULL_CFG)
    return out


# revision 10
# speedup vs baseline: 1.8505x; 1.0851x over previous
"""Trainium2 Bass kernel for a 2-layer edge-featured GAT + mean-pool + FC.

Sharding: 256 graphs split 32-per-core across 8 cores; batch is sorted so each
core owns a contiguous, graph-aligned node range. Edges live on the core that
owns their destination node. Per layer, each core's node table (features +
attention terms) is AllGathered so any core can gather arbitrary source rows.

Node-table rows are bf16: layer-1 rows are 384 bf16 elems (768B) laid out as
[h(256 bf16) | a_src(4 fp32, bitcast) | pad]; layer-2 rows are 128 bf16 elems
(256B) as [h2(64) | a_src2(1 fp32 bitcast) | pad]. Only ONE dma_gather per
128-node destination tile pulls all its (padded) edges' source rows; the
destination-side attention term a_dst needs no DRAM gather at all: since every
edge's destination is one of the tile's 128 local nodes, a transposed one-hot
ohT[node, edge] (built from a uint8 replicated dst-local index row with a
single is_equal) turns the lookup into tiny per-chunk matmuls
ae[e,h] = sum_n ohT[n,e] * a_dst[n,h] on the tensor engine.

Attention weights p = exp(leaky_relu(a_src+a_dst+w*q)) are computed as
max(exp(x), exp(0.2x)), written as bf16 into the gathered rows' pad columns,
messages are h*p, and the per-destination segment-sum is a one-hot matmul
accumulated in PSUM, with p itself carried as extra columns to produce the
softmax denominators. Normalization happens once per node after aggregation
(the reference's max-subtraction cancels exactly in the ratio). Pad edge
slots gather row 0 (finite) and carry dst_local = -1 so their one-hot columns
are all zero and they contribute nothing.
"""

import sys

sys.path.insert(0, "/opt/trn_rl_repo")

import math
from contextlib import ExitStack

import ml_dtypes
import numpy as np

import concourse.bacc as bacc
import concourse.bass as bass
import concourse.mybir as mybir
import concourse.tile as tile
from concourse.bass_utils import run_bass_kernel_spmd
from concourse.masks import make_identity

P = 128
NCORES = 8

FULL_CFG = dict(N=20000, E=640000, FIN=128, HID=64, HEADS=4, NG=256, OUT=32)

F32 = mybir.dt.float32
BF16 = mybir.dt.bfloat16
U8 = mybir.dt.uint8
I16 = mybir.dt.int16

BF = ml_dtypes.bfloat16

ROW1 = 384   # bf16 elems: h(256) | a_src fp32 (8 slots) | pad -> 768B
ROW2 = 128   # bf16 elems: h2(64) | a_src2 fp32 (2 slots) | pad -> 256B


# ---------------------------------------------------------------------------
# Host-side preparation: integer index manipulation + array reordering only.
# ---------------------------------------------------------------------------
def prepare(inputs, cfg):
    N, E, FIN, HID, HEADS, NG, OUT = (
        cfg["N"], cfg["E"], cfg["FIN"], cfg["HID"], cfg["HEADS"], cfg["NG"],
        cfg["OUT"],
    )
    GPC = NG // NCORES  # graphs per core

    x = np.asarray(inputs["x"], np.float32)
    ei = np.asarray(inputs["edge_index"], np.int64)
    ea = np.asarray(inputs["edge_attr"], np.float32)
    batch = np.asarray(inputs["batch"], np.int64)
    src, dst = ei[0], ei[1]

    # node ranges per core (graph-aligned; batch is sorted)
    bounds = np.searchsorted(batch, np.arange(NCORES + 1) * GPC)
    node_cnt = np.diff(bounds)
    NT = max(1, math.ceil(node_cnt.max() / P))
    NSLICE = NT * P
    NROWS = NCORES * NSLICE
    assert NROWS < 32768, f"int16 gather index overflow: {NROWS}"

    core_of_node = np.minimum(batch // GPC, NCORES - 1).astype(np.int64)
    rowid = np.empty(N, np.int64)
    for c in range(NCORES):
        ns, ne = bounds[c], bounds[c + 1]
        rowid[ns:ne] = c * NSLICE + np.arange(ne - ns)

    # edges sorted by dst; since batch is sorted, core blocks are contiguous
    order = np.argsort(dst, kind="stable")
    dsts = dst[order]
    srcs = src[order]
    ws = ea[order, 0]
    ecore = core_of_node[dsts]
    ebounds = np.searchsorted(ecore, np.arange(NCORES + 1))

    # chunks-per-tile: max over all (core, tile)
    cpt_max = 1
    tile_edge_counts = []
    for c in range(NCORES):
        es, ee = ebounds[c], ebounds[c + 1]
        dln = dsts[es:ee] - bounds[c]
        tid = dln // P
        cnts = np.bincount(tid, minlength=NT)
        tile_edge_counts.append(cnts)
        if len(cnts):
            cpt_max = max(cpt_max, math.ceil(cnts.max() / P))
    CPT = cpt_max
    NCHUNK = NT * CPT

    per_core = []
    for c in range(NCORES):
        ns, ne = bounds[c], bounds[c + 1]
        es, ee = ebounds[c], ebounds[c + 1]
        nloc = ne - ns

        xs = np.zeros((NSLICE, FIN), np.float32)
        xs[:nloc] = x[ns:ne]

        gl = np.full((NT * P,), -1.0, np.float32)
        gl[:nloc] = (batch[ns:ne] - c * GPC).astype(np.float32)
        gl_dev = gl.reshape(NT, P).T.copy()  # [128, NT]

        srcrow = np.zeros((NT, CPT * P), np.int64)   # pad -> row 0 (finite)
        dstl = np.full((NT, CPT * P), -1.0, np.float32)
        dstu8 = np.full((NT, CPT * P), 255, np.uint8)
        wv = np.zeros((NT, CPT * P), np.float32)

        dln = dsts[es:ee] - ns
        cnts = tile_edge_counts[c]
        off = np.zeros(NT + 1, np.int64)
        off[1:NT + 1] = np.cumsum(cnts[:NT])
        for t in range(NT):
            k = int(cnts[t]) if t < len(cnts) else 0
            if k == 0:
                continue
            sel = slice(es + int(off[t]), es + int(off[t]) + k)
            srcrow[t, :k] = rowid[srcs[sel]]
            loc = (dln[int(off[t]):int(off[t]) + k] % P)
            dstl[t, :k] = loc.astype(np.float32)
            dstu8[t, :k] = loc.astype(np.uint8)
            wv[t, :k] = ws[sel]

        # device layouts
        dstl_dev = dstl.reshape(NCHUNK, P).T.astype(BF).copy()  # [128, NCHUNK]
        wv_dev = wv.reshape(NCHUNK, P).T.copy()

        def wrap_idx(arr):  # [NT, CPT*P] -> [128, NT*CPT*8] int16
            blocks = []
            for t in range(NT):
                a = arr[t].reshape(CPT * 8, 16).T  # [16, CPT*8]
                blocks.append(np.tile(a, (8, 1)))
            return np.ascontiguousarray(
                np.concatenate(blocks, axis=1)).astype(np.int16)

        per_core.append(dict(
            xs=xs.astype(BF), gl=gl_dev, dstl=dstl_dev, wv=wv_dev,
            du=dstu8[None],                  # [1, NT, CPT*P] uint8
            ixs=wrap_idx(srcrow),
            cnts=np.minimum(cnts[:NT], CPT * P).astype(np.int32)[None],
        ))

    # weight-side constants (tiny, host-replicated)
    W1 = np.asarray(inputs["W1"], np.float32)            # [FIN, H*HID]
    W2 = np.asarray(inputs["W2"], np.float32)            # [H*HID, HID]
    as1 = np.asarray(inputs["att_src1"], np.float32).reshape(-1)
    ad1 = np.asarray(inputs["att_dst1"], np.float32).reshape(-1)
    as2 = np.asarray(inputs["att_src2"], np.float32).reshape(-1)
    ad2 = np.asarray(inputs["att_dst2"], np.float32).reshape(-1)
    q1 = (np.asarray(inputs["We1"], np.float32).reshape(HEADS, HID)
          * np.asarray(inputs["att_edge1"], np.float32)).sum(axis=1)  # [H]
    q2 = float((np.asarray(inputs["We2"], np.float32).reshape(-1)
                * np.asarray(inputs["att_edge2"], np.float32).reshape(-1))
               .sum())
    b1 = np.asarray(inputs["b1"], np.float32)
    b2 = np.asarray(inputs["b2"], np.float32)
    fcW = np.asarray(inputs["fcW"], np.float32)
    fcb = np.asarray(inputs["fcb"], np.float32)

    rep = lambda vv: np.tile(vv[None, :].astype(np.float32), (P, 1)).copy()
    consts = dict(
        W1=W1.astype(BF),
        W2=np.ascontiguousarray(
            W2.reshape(2, P, HID).transpose(1, 0, 2)).astype(BF),  # [P,2,HID]
        as1b=rep(as1), ad1b=rep(ad1), b1b=rep(b1),
        as2b=rep(as2), ad2b=rep(ad2), b2b=rep(b2),
        q1b=rep(q1), q2b=np.full((P, 1), q2, np.float32),
        fcw=fcW, fcbb=rep(fcb),
        iota=np.tile(np.arange(P, dtype=np.float32)[None, :], (P, 1)).copy(),
        iotab=np.tile(np.arange(P, dtype=np.float32)[None, :],
                      (P, 1)).astype(BF),
        iotac=np.arange(P, dtype=np.float32)[:, None].copy(),
    )

    in_maps = []
    for c in range(NCORES):
        m = dict(per_core[c])
        m.update(consts)
        in_maps.append(m)

    meta = dict(NT=NT, CPT=CPT, NSLICE=NSLICE, NROWS=NROWS, GPC=GPC, **cfg)
    return in_maps, meta


# ---------------------------------------------------------------------------
# Device program.
# ---------------------------------------------------------------------------
def build(meta, num_devices=NCORES):
    NT, CPT = meta["NT"], meta["CPT"]
    NSLICE, NROWS, GPC = meta["NSLICE"], meta["NROWS"], meta["GPC"]
    FIN, HID, HEADS, OUT = meta["FIN"], meta["HID"], meta["HEADS"], meta["OUT"]
    D1 = HEADS * HID          # 256
    NI = CPT * P              # gather idxs per tile
    NIc = NI // 16            # idx columns per tile (= CPT*8)
    NCHUNK = NT * CPT
    A = mybir.AluOpType
    ACT = mybir.ActivationFunctionType
    X = mybir.AxisListType.X
    rg = [list(range(NCORES))]

    nc = bacc.Bacc("TRN2", target_bir_lowering=False, debug=False,
                   num_devices=num_devices,
                   dynamic_dma_scratch_size=32768)

    def din(name, shape, dtype=F32):
        return nc.dram_tensor(name, list(shape), dtype,
                              kind="ExternalInput").ap()

    xs = din("xs", (NSLICE, FIN), BF16)
    ixs_d = din("ixs", (P, NT * NIc), I16)
    du_d = din("du", (1, NT, NI), U8)
    dstl_d = din("dstl", (P, NCHUNK), BF16)
    wv_d = din("wv", (P, NCHUNK))
    gl_d = din("gl", (P, NT))
    W1_d = din("W1", (FIN, D1), BF16)
    W2_d = din("W2", (P, 2, HID), BF16)
    as1_d = din("as1b", (P, D1))
    ad1_d = din("ad1b", (P, D1))
    b1_d = din("b1b", (P, D1))
    as2_d = din("as2b", (P, HID))
    ad2_d = din("ad2b", (P, HID))
    b2_d = din("b2b", (P, HID))
    q1_d = din("q1b", (P, HEADS))
    q2_d = din("q2b", (P, 1))
    fcw_d = din("fcw", (HID, OUT))
    fcb_d = din("fcbb", (P, OUT))
    iota_d = din("iota", (P, P))
    iotab_d = din("iotab", (P, P), BF16)
    iotac_d = din("iotac", (P, 1))
    cnts_d = din("cnts", (1, NT), mybir.dt.int32)

    out_d = nc.dram_tensor("out", [GPC, OUT], F32, kind="ExternalOutput").ap()

    with tile.TileContext(nc) as tc, ExitStack() as st:
        constp = st.enter_context(tc.tile_pool(name="constp", bufs=1))
        drp = st.enter_context(tc.tile_pool(name="drp", bufs=1, space="DRAM"))

        # whole-kernel constants
        iota_sb = constp.tile([P, P], F32)
        nc.sync.dma_start(iota_sb[:], iota_d[:])
        iotab_sb = constp.tile([P, P], BF16)
        nc.sync.dma_start(iotab_sb[:], iotab_d[:])
        iotac_sb = constp.tile([P, 1], F32)
        nc.sync.dma_start(iotac_sb[:], iotac_d[:])
        identf = constp.tile([P, P], F32)
        make_identity(nc, identf[:])
        identb = constp.tile([P, P], BF16)
        make_identity(nc, identb[:])
        dstl_sb = constp.tile([P, NCHUNK], BF16)
        nc.sync.dma_start(dstl_sb[:], dstl_d[:])
        wv_sb = constp.tile([P, NCHUNK], F32)
        nc.sync.dma_start(wv_sb[:], wv_d[:])
        gl_sb = constp.tile([P, NT], F32)
        nc.sync.dma_start(gl_sb[:], gl_d[:])
        q1_sb = constp.tile([P, HEADS], F32)
        nc.sync.dma_start(q1_sb[:], q1_d[:])
        q2_sb = constp.tile([P, 1], F32)
        nc.sync.dma_start(q2_sb[:], q2_d[:])
        ixs_all = constp.tile([P, NT * NIc], I16)
        nc.sync.dma_start(ixs_all[:], ixs_d[:])
        cnts_sb = constp.tile([1, NT], mybir.dt.int32)
        nc.sync.dma_start(cnts_sb[:], cnts_d[:])
        adst_all = constp.tile([P, NT, HEADS], BF16)
        adst2_all = constp.tile([P, NT], BF16)
        out1 = constp.tile([P, NT * D1], BF16)

        t1loc = drp.tile([NSLICE, ROW1], BF16, name="t1loc")
        t1full = drp.tile([NROWS, ROW1], BF16, addr_space="Shared",
                          name="t1full")
        t2loc = drp.tile([NSLICE, ROW2], BF16, name="t2loc")
        t2full = drp.tile([NROWS, ROW2], BF16, addr_space="Shared",
                          name="t2full")

        # ---------------- Phase 0: h1 = x @ W1, a_src/a_dst, table1 ---------
        with tc.tile_pool(name="ph0", bufs=1) as sp, \
             tc.tile_pool(name="ph0b", bufs=2) as sp2, \
             tc.tile_pool(name="ph0p", bufs=2, space="PSUM") as pp:
            w1_sb = sp.tile([P, D1], BF16)
            nc.sync.dma_start(w1_sb[:], W1_d[:])
            as1_sb = sp.tile([P, D1], F32)
            nc.sync.dma_start(as1_sb[:], as1_d[:])
            ad1_sb = sp.tile([P, D1], F32)
            nc.sync.dma_start(ad1_sb[:], ad1_d[:])
            xall = sp.tile([P, NT, FIN], BF16)
            nc.sync.dma_start(xall[:],
                              xs[:].rearrange("(t p) f -> p t f", p=P))
            for t in range(NT):
                xT_ps = pp.tile([P, P], BF16, space="PSUM")
                nc.tensor.transpose(xT_ps[:], xall[:, t, :], identb[:])
                xT = sp2.tile([P, P], BF16)
                nc.vector.tensor_copy(out=xT[:], in_=xT_ps[:])
                h_ps = pp.tile([P, D1], F32, space="PSUM")
                nc.tensor.matmul(h_ps[:], lhsT=xT[:], rhs=w1_sb[:],
                                 start=True, stop=True)
                t1t = sp2.tile([P, ROW1], BF16)
                tmp = sp2.tile([P, D1], F32)
                nc.vector.tensor_tensor(out=tmp[:], in0=h_ps[:],
                                        in1=as1_sb[:], op=A.mult)
                nc.vector.tensor_reduce(
                    out=t1t[:, D1:D1 + 2 * HEADS].bitcast(F32),
                    in_=tmp[:].rearrange("p (h f) -> p h f", h=HEADS),
                    axis=X, op=A.add)
                nc.vector.tensor_tensor(out=tmp[:], in0=h_ps[:],
                                        in1=ad1_sb[:], op=A.mult)
                adf = sp2.tile([P, HEADS], F32)
                nc.vector.tensor_reduce(
                    out=adf[:],
                    in_=tmp[:].rearrange("p (h f) -> p h f", h=HEADS),
                    axis=X, op=A.add)
                nc.vector.tensor_copy(out=adst_all[:, t, :], in_=adf[:])
                nc.vector.tensor_copy(out=t1t[:, 0:D1], in_=h_ps[:])
                nc.vector.memset(t1t[:, D1 + 2 * HEADS:ROW1], 0.0)
                nc.sync.dma_start(t1loc[t * P:(t + 1) * P, :], t1t[:])
            nc.gpsimd.collective_compute(
                "AllGather", A.bypass, replica_groups=rg,
                ins=[t1loc[:]], outs=[t1full[:]])

        # ---------------- Phase 1: layer-1 edge phase -----------------------
        with tc.tile_pool(name="p1g", bufs=3) as pg, \
             tc.tile_pool(name="p1u", bufs=2) as pu, \
             tc.tile_pool(name="p1o", bufs=2) as po, \
             tc.tile_pool(name="p1w", bufs=3) as pw, \
             tc.tile_pool(name="p1e", bufs=1) as pe, \
             tc.tile_pool(name="p1b", bufs=2) as pb, \
             tc.tile_pool(name="p1ps", bufs=2, space="PSUM") as pps, \
             tc.tile_pool(name="p1pa", bufs=2, space="PSUM") as ppa:
            b1_sb = pe.tile([P, D1], F32)
            nc.sync.dma_start(b1_sb[:], b1_d[:])
            w2_sb = pe.tile([P, 2, HID], BF16)
            nc.sync.dma_start(w2_sb[:], W2_d[:])
            as2_sb = pe.tile([P, HID], F32)
            nc.sync.dma_start(as2_sb[:], as2_d[:])
            ad2_sb = pe.tile([P, HID], F32)
            nc.sync.dma_start(ad2_sb[:], ad2_d[:])
            for t in range(NT):
                G = pg.tile([P, CPT, ROW1], BF16)
                if t < 3:
                    nc.vector.memset(G[:], 0.0)
                nc.gpsimd.dma_gather(
                    G[:], t1full[:], ixs_all[:, t * NIc:(t + 1) * NIc],
                    NI, NI, ROW1, single_packet=False)
                du = pu.tile([P, NI], U8)
                nc.sync.dma_start(du[:],
                                  du_d[0:1, t, :].partition_broadcast(P))
                ohT = po.tile([P, NI], BF16)
                nc.vector.tensor_scalar(out=ohT[:], in0=du[:],
                                        scalar1=iotac_sb[:, 0:1],
                                        scalar2=None, op0=A.is_equal)
                ae_ps = ppa.tile([P, CPT, HEADS], F32, space="PSUM")
                for c in range(CPT):
                    nc.tensor.matmul(ae_ps[:, c, :],
                                     lhsT=ohT[:, c * P:(c + 1) * P],
                                     rhs=adst_all[:, t, :],
                                     start=True, stop=True)
                oh = po.tile([P, CPT, P], BF16)
                nc.vector.tensor_tensor(
                    out=oh[:],
                    in0=iotab_sb[:].unsqueeze(1).to_broadcast([P, CPT, P]),
                    in1=dstl_sb[:, t * CPT:(t + 1) * CPT]
                        .unsqueeze(2).to_broadcast([P, CPT, P]),
                    op=A.is_equal)
                # alpha = a_src + a_dst + w*q ; p = max(exp(a), exp(0.2a))
                asr = pw.tile([P, CPT, HEADS], F32)
                nc.vector.tensor_tensor(
                    out=asr[:],
                    in0=wv_sb[:, t * CPT:(t + 1) * CPT].unsqueeze(2)
                        .to_broadcast([P, CPT, HEADS]),
                    in1=q1_sb[:].unsqueeze(1).to_broadcast([P, CPT, HEADS]),
                    op=A.mult)
                nc.vector.tensor_tensor(
                    out=asr[:], in0=asr[:],
                    in1=G[:, :, D1:D1 + 2 * HEADS].bitcast(F32), op=A.add)
                nc.vector.tensor_tensor(out=asr[:], in0=asr[:], in1=ae_ps[:],
                                        op=A.add)
                e2 = pw.tile([P, CPT, HEADS], F32)
                nc.scalar.activation(out=e2[:], in_=asr[:], func=ACT.Exp,
                                     scale=0.2)
                nc.scalar.activation(out=asr[:], in_=asr[:], func=ACT.Exp)
                pbf = G[:, :, D1:D1 + HEADS]   # bf16 p slot
                nc.vector.tensor_tensor(out=pbf, in0=asr[:], in1=e2[:],
                                        op=A.max)
                gm = G[:, :, 0:D1].rearrange("p c (h f) -> p c h f", h=HEADS)
                nc.vector.tensor_tensor(
                    out=gm, in0=gm,
                    in1=pbf.unsqueeze(3).to_broadcast([P, CPT, HEADS, HID]),
                    op=A.mult)
                acc = pps.tile([P, D1 + HEADS], F32, space="PSUM")
                for c in range(CPT):
                    nc.tensor.matmul(acc[:], lhsT=oh[:, c, :],
                                     rhs=G[:, c, 0:D1 + HEADS],
                                     start=(c == 0), stop=(c == CPT - 1))
                # epilogue: out1 = relu(acc/denom + b1)
                dn = pw.tile([P, HEADS], F32)
                nc.vector.tensor_scalar(out=dn[:], in0=acc[:, D1:D1 + HEADS],
                                        scalar1=1e-16, scalar2=None,
                                        op0=A.add)
                rc = pw.tile([P, HEADS], F32)
                nc.vector.reciprocal(rc[:], dn[:])
                ob = out1[:, t * D1:(t + 1) * D1]
                nc.vector.tensor_tensor(
                    out=ob.rearrange("p (h f) -> p h f", h=HEADS),
                    in0=acc[:, 0:D1].rearrange("p (h f) -> p h f", h=HEADS),
                    in1=rc[:].unsqueeze(2).to_broadcast([P, HEADS, HID]),
                    op=A.mult)
                nc.vector.tensor_tensor(out=ob, in0=ob, in1=b1_sb[:],
                                        op=A.add)
                nc.vector.tensor_scalar(out=ob, in0=ob, scalar1=0.0,
                                        scalar2=None, op0=A.max)
                # ---- layer-2 row for this tile (h2, a_src2, a_dst2) --------
                h2_ps = ppa.tile([P, HID], F32, space="PSUM")
                for k in range(2):
                    hT_ps = ppa.tile([P, P], BF16, space="PSUM")
                    nc.tensor.transpose(
                        hT_ps[:],
                        out1[:, t * D1 + k * P:t * D1 + (k + 1) * P],
                        identb[:])
                    hT = pb.tile([P, P], BF16)
                    nc.vector.tensor_copy(out=hT[:], in_=hT_ps[:])
                    nc.tensor.matmul(h2_ps[:], lhsT=hT[:],
                                     rhs=w2_sb[:, k, :],
                                     start=(k == 0), stop=(k == 1))
                t2t = pb.tile([P, ROW2], BF16)
                tmp = pb.tile([P, HID], F32)
                nc.vector.tensor_tensor(out=tmp[:], in0=h2_ps[:],
                                        in1=as2_sb[:], op=A.mult)
                nc.vector.tensor_reduce(out=t2t[:, HID:HID + 2].bitcast(F32),
                                        in_=tmp[:], axis=X, op=A.add)
                nc.vector.tensor_tensor(out=tmp[:], in0=h2_ps[:],
                                        in1=ad2_sb[:], op=A.mult)
                ad2f = pb.tile([P, 1], F32)
                nc.vector.tensor_reduce(out=ad2f[:], in_=tmp[:], axis=X,
                                        op=A.add)
                nc.vector.tensor_copy(out=adst2_all[:, t:t + 1], in_=ad2f[:])
                nc.vector.tensor_copy(out=t2t[:, 0:HID], in_=h2_ps[:])
                nc.vector.memset(t2t[:, HID + 2:ROW2], 0.0)
                nc.sync.dma_start(t2loc[t * P:(t + 1) * P, :], t2t[:])
            nc.gpsimd.collective_compute(
                "AllGather", A.bypass, replica_groups=rg,
                ins=[t2loc[:]], outs=[t2full[:]])

        # ---------------- Phase 3: layer-2 edge phase + pooling -------------
        with tc.tile_pool(name="p3g", bufs=4) as pg, \
             tc.tile_pool(name="p3u", bufs=2) as pu, \
             tc.tile_pool(name="p3o", bufs=3) as po, \
             tc.tile_pool(name="p3w", bufs=3) as pw, \
             tc.tile_pool(name="p3c", bufs=1) as pc, \
             tc.tile_pool(name="p3ps", bufs=2, space="PSUM") as pps, \
             tc.tile_pool(name="p3pa", bufs=2, space="PSUM") as ppa, \
             tc.tile_pool(name="p3pl", bufs=1, space="PSUM") as ppl:
            b2_sb = pc.tile([P, HID], F32)
            nc.sync.dma_start(b2_sb[:], b2_d[:])
            pool_ps = ppl.tile([GPC, HID + 1], F32, space="PSUM")
            for t in range(NT):
                G = pg.tile([P, CPT, ROW2], BF16)
                if t < 4:
                    nc.vector.memset(G[:], 0.0)
                nc.gpsimd.dma_gather(
                    G[:], t2full[:], ixs_all[:, t * NIc:(t + 1) * NIc],
                    NI, NI, ROW2, single_packet=False)
                du = pu.tile([P, NI], U8)
                nc.sync.dma_start(du[:],
                                  du_d[0:1, t, :].partition_broadcast(P))
                ohT = po.tile([P, NI], BF16)
                nc.vector.tensor_scalar(out=ohT[:], in0=du[:],
                                        scalar1=iotac_sb[:, 0:1],
                                        scalar2=None, op0=A.is_equal)
                ae_ps = ppa.tile([P, CPT, 1], F32, space="PSUM")
                for c in range(CPT):
                    nc.tensor.matmul(ae_ps[:, c, :],
                                     lhsT=ohT[:, c * P:(c + 1) * P],
                                     rhs=adst2_all[:, t:t + 1],
                                     start=True, stop=True)
                oh = po.tile([P, CPT, P], BF16)
                nc.vector.tensor_tensor(
                    out=oh[:],
                    in0=iotab_sb[:].unsqueeze(1).to_broadcast([P, CPT, P]),
                    in1=dstl_sb[:, t * CPT:(t + 1) * CPT]
                        .unsqueeze(2).to_broadcast([P, CPT, P]),
                    op=A.is_equal)
                asr = pw.tile([P, CPT, 1], F32)
                nc.vector.tensor_scalar(
                    out=asr[:], in0=wv_sb[:, t * CPT:(t + 1) * CPT]
                        .unsqueeze(2),
                    scalar1=q2_sb[:, 0:1], scalar2=None, op0=A.mult)
                nc.vector.tensor_tensor(
                    out=asr[:], in0=asr[:],
                    in1=G[:, :, HID:HID + 2].bitcast(F32), op=A.add)
                nc.vector.tensor_tensor(out=asr[:], in0=asr[:], in1=ae_ps[:],
                                        op=A.add)
                e2 = pw.tile([P, CPT, 1], F32)
                nc.scalar.activation(out=e2[:], in_=asr[:], func=ACT.Exp,
                                     scale=0.2)
                nc.scalar.activation(out=asr[:], in_=asr[:], func=ACT.Exp)
                pbf = G[:, :, HID:HID + 1]
                nc.vector.tensor_tensor(out=pbf, in0=asr[:], in1=e2[:],
                                        op=A.max)
                gm = G[:, :, 0:HID]
                nc.vector.tensor_tensor(
                    out=gm, in0=gm,
                    in1=pbf.to_broadcast([P, CPT, HID]), op=A.mult)
                acc = pps.tile([P, HID + 1], F32, space="PSUM")
                for c in range(CPT):
                    nc.tensor.matmul(acc[:], lhsT=oh[:, c, :],
                                     rhs=G[:, c, 0:HID + 1],
                                     start=(c == 0), stop=(c == CPT - 1))
                # epilogue: o2 = [relu(acc/denom + b2) | 1], pool matmul
                dn = pw.tile([P, 1], F32)
                nc.vector.tensor_scalar(out=dn[:], in0=acc[:, HID:HID + 1],
                                        scalar1=1e-16, scalar2=None,
                                        op0=A.add)
                rc = pw.tile([P, 1], F32)
                nc.vector.reciprocal(rc[:], dn[:])
                o2 = pw.tile([P, HID + 1], F32)
                nc.vector.tensor_scalar(out=o2[:, 0:HID], in0=acc[:, 0:HID],
                                        scalar1=rc[:, 0:1], scalar2=None,
                                        op0=A.mult)
                nc.vector.tensor_tensor(out=o2[:, 0:HID], in0=o2[:, 0:HID],
                                        in1=b2_sb[:], op=A.add)
                nc.vector.tensor_scalar(out=o2[:, 0:HID], in0=o2[:, 0:HID],
                                        scalar1=0.0, scalar2=None, op0=A.max)
                nc.vector.memset(o2[:, HID:HID + 1], 1.0)
                ohg = pw.tile([P, GPC], F32)
                nc.vector.tensor_scalar(
                    out=ohg[:], in0=iota_sb[:, 0:GPC],
                    scalar1=gl_sb[:, t:t + 1], scalar2=None, op0=A.is_equal)
                nc.tensor.matmul(pool_ps[:], lhsT=ohg[:], rhs=o2[:],
                                 start=(t == 0), stop=(t == NT - 1),
                                 skip_group_check=True)

            # ------------- Phase 4: pooled mean + FC ------------------------
            fcw_sb = pc.tile([HID, OUT], F32)
            nc.sync.dma_start(fcw_sb[:], fcw_d[:])
            fcb_sb = pc.tile([P, OUT], F32)
            nc.sync.dma_start(fcb_sb[:], fcb_d[:])
            cnt = pc.tile([GPC, 1], F32)
            nc.vector.tensor_scalar(out=cnt[:], in0=pool_ps[:, HID:HID + 1],
                                    scalar1=1.0, scalar2=None, op0=A.max)
            rcc = pc.tile([GPC, 1], F32)
            nc.vector.reciprocal(rcc[:], cnt[:])
            pooled = pc.tile([GPC, HID], F32)
            nc.vector.tensor_scalar(out=pooled[:], in0=pool_ps[:, 0:HID],
                                    scalar1=rcc[:, 0:1], scalar2=None,
                                    op0=A.mult)
            pT_ps = ppl.tile([HID, GPC], F32, space="PSUM")
            nc.tensor.transpose(pT_ps[:], pooled[:], identf[:GPC, :GPC])
            pT = pc.tile([HID, GPC], F32)
            nc.vector.tensor_copy(out=pT[:], in_=pT_ps[:])
            fc_ps = ppl.tile([GPC, OUT], F32, space="PSUM")
            nc.tensor.matmul(fc_ps[:], lhsT=pT[:], rhs=fcw_sb[:],
                             start=True, stop=True)
            res = pc.tile([GPC, OUT], F32)
            nc.vector.tensor_tensor(out=res[:], in0=fc_ps[:],
                                    in1=fcb_sb[:GPC, :], op=A.add)
            nc.sync.dma_start(out_d[:], res[:])

    nc.compile()
    return nc


# ---------------------------------------------------------------------------
# Entry point.
# ---------------------------------------------------------------------------
def run(inputs, cfg, **run_kwargs):
    in_maps, meta = prepare(inputs, cfg)
    nc = build(meta)
    res = run_bass_kernel_spmd(nc, in_maps, core_ids=list(range(NCORES)),
                               **run_kwargs)
    out = np.concatenate([res.results[c]["out"] for c in range(NCORES)],
                         axis=0)
    return np.asarray(out, np.float32), res


def kernel(**inputs) -> np.ndarray:
    out, _ = run(inputs, FULL_CFG)
    return out


# revision 12
# speedup vs baseline: 1.9199x; 1.0375x over previous
"""Trainium2 Bass kernel for a 2-layer edge-featured GAT + mean-pool + FC.

Sharding: 256 graphs split 32-per-core across 8 cores; batch is sorted so each
core owns a contiguous, graph-aligned node range. Edges live on the core that
owns their destination node. Per layer, each core's node table (features +
attention terms) is AllGathered so any core can gather arbitrary source rows.

Node-table rows are bf16: layer-1 rows are 384 bf16 elems (768B) laid out as
[h(256 bf16) | a_src(4 fp32, bitcast) | pad]; layer-2 rows are 128 bf16 elems
(256B) as [h2(64) | a_src2(1 fp32 bitcast) | pad]. Only ONE dma_gather per
128-node destination tile pulls all its (padded) edges' source rows; the
destination-side attention term a_dst needs no DRAM gather at all: since every
edge's destination is one of the tile's 128 local nodes, a transposed one-hot
ohT[node, edge] (built from a uint8 replicated dst-local index row with a
single is_equal) turns the lookup into tiny per-chunk matmuls
ae[e,h] = sum_n ohT[n,e] * a_dst[n,h] on the tensor engine.

Attention weights p = exp(leaky_relu(a_src+a_dst+w*q)) are computed as
max(exp(x), exp(0.2x)), written as bf16 into the gathered rows' pad columns,
messages are h*p, and the per-destination segment-sum is a one-hot matmul
accumulated in PSUM, with p itself carried as extra columns to produce the
softmax denominators. Normalization happens once per node after aggregation
(the reference's max-subtraction cancels exactly in the ratio). Pad edge
slots gather row 0 (finite) and carry dst_local = -1 so their one-hot columns
are all zero and they contribute nothing.
"""

import sys

sys.path.insert(0, "/opt/trn_rl_repo")

import math
from contextlib import ExitStack

import ml_dtypes
import numpy as np

import concourse.bacc as bacc
import concourse.bass as bass
import concourse.mybir as mybir
import concourse.tile as tile
from concourse.bass_utils import run_bass_kernel_spmd
from concourse.masks import make_identity

P = 128
NCORES = 8

FULL_CFG = dict(N=20000, E=640000, FIN=128, HID=64, HEADS=4, NG=256, OUT=32)

F32 = mybir.dt.float32
BF16 = mybir.dt.bfloat16
U8 = mybir.dt.uint8
I16 = mybir.dt.int16

BF = ml_dtypes.bfloat16

ROW1 = 384   # bf16 elems: h(256) | a_src fp32 (8 slots) | pad -> 768B
ROW2 = 128   # bf16 elems: h2(64) | a_src2 fp32 (2 slots) | pad -> 256B


# ---------------------------------------------------------------------------
# Host-side preparation: integer index manipulation + array reordering only.
# ---------------------------------------------------------------------------
def prepare(inputs, cfg):
    N, E, FIN, HID, HEADS, NG, OUT = (
        cfg["N"], cfg["E"], cfg["FIN"], cfg["HID"], cfg["HEADS"], cfg["NG"],
        cfg["OUT"],
    )
    GPC = NG // NCORES  # graphs per core

    x = np.asarray(inputs["x"], np.float32)
    ei = np.asarray(inputs["edge_index"], np.int64)
    ea = np.asarray(inputs["edge_attr"], np.float32)
    batch = np.asarray(inputs["batch"], np.int64)
    src, dst = ei[0], ei[1]

    # node ranges per core (graph-aligned; batch is sorted)
    bounds = np.searchsorted(batch, np.arange(NCORES + 1) * GPC)
    node_cnt = np.diff(bounds)
    NT = max(1, math.ceil(node_cnt.max() / P))
    NSLICE = NT * P
    NROWS = NCORES * NSLICE
    assert NROWS < 32768, f"int16 gather index overflow: {NROWS}"

    core_of_node = np.minimum(batch // GPC, NCORES - 1).astype(np.int64)
    rowid = np.empty(N, np.int64)
    H2 = max(1, NT - 5)  # tiles in the first (early) AllGather half
    rowid2 = np.empty(N, np.int64)
    for c in range(NCORES):
        ns, ne = bounds[c], bounds[c + 1]
        loc = np.arange(ne - ns)
        rowid[ns:ne] = c * NSLICE + loc
        t, sl = loc // P, loc % P
        rowid2[ns:ne] = np.where(
            t < H2,
            c * H2 * P + t * P + sl,
            NCORES * H2 * P + c * (NT - H2) * P + (t - H2) * P + sl)

    # edges sorted by dst; since batch is sorted, core blocks are contiguous
    order = np.argsort(dst, kind="stable")
    dsts = dst[order]
    srcs = src[order]
    ws = ea[order, 0]
    ecore = core_of_node[dsts]
    ebounds = np.searchsorted(ecore, np.arange(NCORES + 1))

    # chunks-per-tile: max over all (core, tile)
    cpt_max = 1
    tile_edge_counts = []
    for c in range(NCORES):
        es, ee = ebounds[c], ebounds[c + 1]
        dln = dsts[es:ee] - bounds[c]
        tid = dln // P
        cnts = np.bincount(tid, minlength=NT)
        tile_edge_counts.append(cnts)
        if len(cnts):
            cpt_max = max(cpt_max, math.ceil(cnts.max() / P))
    CPT = cpt_max
    NCHUNK = NT * CPT

    per_core = []
    for c in range(NCORES):
        ns, ne = bounds[c], bounds[c + 1]
        es, ee = ebounds[c], ebounds[c + 1]
        nloc = ne - ns

        xs = np.zeros((NSLICE, FIN), np.float32)
        xs[:nloc] = x[ns:ne]

        gl = np.full((NT * P,), -1.0, np.float32)
        gl[:nloc] = (batch[ns:ne] - c * GPC).astype(np.float32)
        gl_dev = gl.reshape(NT, P).T.copy()  # [128, NT]

        srcrow = np.zeros((NT, CPT * P), np.int64)   # pad -> row 0 (finite)
        srcrow2 = np.zeros((NT, CPT * P), np.int64)
        dstl = np.full((NT, CPT * P), -1.0, np.float32)
        dstu8 = np.full((NT, CPT * P), 255, np.uint8)
        wv = np.zeros((NT, CPT * P), np.float32)

        dln = dsts[es:ee] - ns
        cnts = tile_edge_counts[c]
        off = np.zeros(NT + 1, np.int64)
        off[1:NT + 1] = np.cumsum(cnts[:NT])
        for t in range(NT):
            k = int(cnts[t]) if t < len(cnts) else 0
            if k == 0:
                continue
            sel = slice(es + int(off[t]), es + int(off[t]) + k)
            srcrow[t, :k] = rowid[srcs[sel]]
            srcrow2[t, :k] = rowid2[srcs[sel]]
            loc = (dln[int(off[t]):int(off[t]) + k] % P)
            dstl[t, :k] = loc.astype(np.float32)
            dstu8[t, :k] = loc.astype(np.uint8)
            wv[t, :k] = ws[sel]

        # device layouts
        dstl_dev = dstl.reshape(NCHUNK, P).T.astype(BF).copy()  # [128, NCHUNK]
        wv_dev = wv.reshape(NCHUNK, P).T.copy()

        def wrap_idx(arr):  # [NT, CPT*P] -> [128, NT*CPT*8] int16
            blocks = []
            for t in range(NT):
                a = arr[t].reshape(CPT * 8, 16).T  # [16, CPT*8]
                blocks.append(np.tile(a, (8, 1)))
            return np.ascontiguousarray(
                np.concatenate(blocks, axis=1)).astype(np.int16)

        per_core.append(dict(
            xs=xs.astype(BF), gl=gl_dev, dstl=dstl_dev, wv=wv_dev,
            du=dstu8[None],                  # [1, NT, CPT*P] uint8
            ixs=wrap_idx(srcrow), ixs2=wrap_idx(srcrow2),
            cnts=np.minimum(cnts[:NT], CPT * P).astype(np.int32)[None],
        ))

    # weight-side constants (tiny, host-replicated)
    W1 = np.asarray(inputs["W1"], np.float32)            # [FIN, H*HID]
    W2 = np.asarray(inputs["W2"], np.float32)            # [H*HID, HID]
    as1 = np.asarray(inputs["att_src1"], np.float32).reshape(-1)
    ad1 = np.asarray(inputs["att_dst1"], np.float32).reshape(-1)
    as2 = np.asarray(inputs["att_src2"], np.float32).reshape(-1)
    ad2 = np.asarray(inputs["att_dst2"], np.float32).reshape(-1)
    q1 = (np.asarray(inputs["We1"], np.float32).reshape(HEADS, HID)
          * np.asarray(inputs["att_edge1"], np.float32)).sum(axis=1)  # [H]
    q2 = float((np.asarray(inputs["We2"], np.float32).reshape(-1)
                * np.asarray(inputs["att_edge2"], np.float32).reshape(-1))
               .sum())
    b1 = np.asarray(inputs["b1"], np.float32)
    b2 = np.asarray(inputs["b2"], np.float32)
    fcW = np.asarray(inputs["fcW"], np.float32)
    fcb = np.asarray(inputs["fcb"], np.float32)

    rep = lambda vv: np.tile(vv[None, :].astype(np.float32), (P, 1)).copy()
    consts = dict(
        W1=W1.astype(BF),
        W2=np.ascontiguousarray(
            W2.reshape(2, P, HID).transpose(1, 0, 2)).astype(BF),  # [P,2,HID]
        as1b=rep(as1), ad1b=rep(ad1), b1b=rep(b1),
        as2b=rep(as2), ad2b=rep(ad2), b2b=rep(b2),
        q1b=rep(q1), q2b=np.full((P, 1), q2, np.float32),
        fcw=fcW, fcbb=rep(fcb),
        iota=np.tile(np.arange(P, dtype=np.float32)[None, :], (P, 1)).copy(),
        iotab=np.tile(np.arange(P, dtype=np.float32)[None, :],
                      (P, 1)).astype(BF),
        iotac=np.arange(P, dtype=np.float32)[:, None].copy(),
    )

    in_maps = []
    for c in range(NCORES):
        m = dict(per_core[c])
        m.update(consts)
        in_maps.append(m)

    meta = dict(NT=NT, CPT=CPT, NSLICE=NSLICE, NROWS=NROWS, GPC=GPC,
                H2=H2, **cfg)
    return in_maps, meta


# ---------------------------------------------------------------------------
# Device program.
# ---------------------------------------------------------------------------
def build(meta, num_devices=NCORES):
    NT, CPT = meta["NT"], meta["CPT"]
    H2 = meta["H2"]
    NSLICE, NROWS, GPC = meta["NSLICE"], meta["NROWS"], meta["GPC"]
    FIN, HID, HEADS, OUT = meta["FIN"], meta["HID"], meta["HEADS"], meta["OUT"]
    D1 = HEADS * HID          # 256
    NI = CPT * P              # gather idxs per tile
    NIc = NI // 16            # idx columns per tile (= CPT*8)
    NCHUNK = NT * CPT
    A = mybir.AluOpType
    ACT = mybir.ActivationFunctionType
    X = mybir.AxisListType.X
    rg = [list(range(NCORES))]

    nc = bacc.Bacc("TRN2", target_bir_lowering=False, debug=False,
                   num_devices=num_devices,
                   dynamic_dma_scratch_size=32768)

    def din(name, shape, dtype=F32):
        return nc.dram_tensor(name, list(shape), dtype,
                              kind="ExternalInput").ap()

    xs = din("xs", (NSLICE, FIN), BF16)
    ixs_d = din("ixs", (P, NT * NIc), I16)
    ixs2_d = din("ixs2", (P, NT * NIc), I16)
    du_d = din("du", (1, NT, NI), U8)
    dstl_d = din("dstl", (P, NCHUNK), BF16)
    wv_d = din("wv", (P, NCHUNK))
    gl_d = din("gl", (P, NT))
    W1_d = din("W1", (FIN, D1), BF16)
    W2_d = din("W2", (P, 2, HID), BF16)
    as1_d = din("as1b", (P, D1))
    ad1_d = din("ad1b", (P, D1))
    b1_d = din("b1b", (P, D1))
    as2_d = din("as2b", (P, HID))
    ad2_d = din("ad2b", (P, HID))
    b2_d = din("b2b", (P, HID))
    q1_d = din("q1b", (P, HEADS))
    q2_d = din("q2b", (P, 1))
    fcw_d = din("fcw", (HID, OUT))
    fcb_d = din("fcbb", (P, OUT))
    iota_d = din("iota", (P, P))
    iotab_d = din("iotab", (P, P), BF16)
    iotac_d = din("iotac", (P, 1))
    cnts_d = din("cnts", (1, NT), mybir.dt.int32)

    out_d = nc.dram_tensor("out", [GPC, OUT], F32, kind="ExternalOutput").ap()

    with tile.TileContext(nc) as tc, ExitStack() as st:
        constp = st.enter_context(tc.tile_pool(name="constp", bufs=1))
        drp = st.enter_context(tc.tile_pool(name="drp", bufs=1, space="DRAM"))

        # whole-kernel constants
        iota_sb = constp.tile([P, P], F32)
        nc.sync.dma_start(iota_sb[:], iota_d[:])
        iotab_sb = constp.tile([P, P], BF16)
        nc.sync.dma_start(iotab_sb[:], iotab_d[:])
        iotac_sb = constp.tile([P, 1], F32)
        nc.sync.dma_start(iotac_sb[:], iotac_d[:])
        identf = constp.tile([P, P], F32)
        make_identity(nc, identf[:])
        identb = constp.tile([P, P], BF16)
        make_identity(nc, identb[:])
        dstl_sb = constp.tile([P, NCHUNK], BF16)
        nc.sync.dma_start(dstl_sb[:], dstl_d[:])
        wv_sb = constp.tile([P, NCHUNK], F32)
        nc.sync.dma_start(wv_sb[:], wv_d[:])
        gl_sb = constp.tile([P, NT], F32)
        nc.sync.dma_start(gl_sb[:], gl_d[:])
        q1_sb = constp.tile([P, HEADS], F32)
        nc.sync.dma_start(q1_sb[:], q1_d[:])
        q2_sb = constp.tile([P, 1], F32)
        nc.sync.dma_start(q2_sb[:], q2_d[:])
        ixs_all = constp.tile([P, NT * NIc], I16)
        nc.sync.dma_start(ixs_all[:], ixs_d[:])

        cnts_sb = constp.tile([1, NT], mybir.dt.int32)
        nc.sync.dma_start(cnts_sb[:], cnts_d[:])
        adst_all = constp.tile([P, NT, HEADS], BF16)
        adst2_all = constp.tile([P, NT], BF16)
        out1 = constp.tile([P, NT * D1], BF16)

        t1loc = drp.tile([NSLICE, ROW1], BF16, name="t1loc")
        t1full = drp.tile([NROWS, ROW1], BF16, addr_space="Shared",
                          name="t1full")
        t2loc = drp.tile([NSLICE, ROW2], BF16, name="t2loc")
        t2full = drp.tile([NROWS, ROW2], BF16, addr_space="Shared",
                          name="t2full")

        # ---------------- Phase 0: h1 = x @ W1, a_src/a_dst, table1 ---------
        with tc.tile_pool(name="ph0", bufs=1) as sp, \
             tc.tile_pool(name="ph0b", bufs=3) as sp2, \
             tc.tile_pool(name="ph0p", bufs=3, space="PSUM") as pp:
            w1_sb = sp.tile([P, D1], BF16)
            nc.sync.dma_start(w1_sb[:], W1_d[:])
            as1_sb = sp.tile([P, D1], F32)
            nc.sync.dma_start(as1_sb[:], as1_d[:])
            ad1_sb = sp.tile([P, D1], F32)
            nc.sync.dma_start(ad1_sb[:], ad1_d[:])
            xall = sp.tile([P, NT, FIN], BF16)
            nc.sync.dma_start(xall[:],
                              xs[:].rearrange("(t p) f -> p t f", p=P))
            for t in range(NT):
                xT_ps = pp.tile([P, P], BF16, space="PSUM")
                nc.tensor.transpose(xT_ps[:], xall[:, t, :], identb[:])
                xT = sp2.tile([P, P], BF16)
                nc.vector.tensor_copy(out=xT[:], in_=xT_ps[:])
                h_ps = pp.tile([P, D1], F32, space="PSUM")
                nc.tensor.matmul(h_ps[:], lhsT=xT[:], rhs=w1_sb[:],
                                 start=True, stop=True)
                t1t = sp2.tile([P, ROW1], BF16)
                tmp = sp2.tile([P, D1], F32)
                nc.vector.tensor_tensor(out=tmp[:], in0=h_ps[:],
                                        in1=as1_sb[:], op=A.mult)
                nc.vector.tensor_reduce(
                    out=t1t[:, D1:D1 + 2 * HEADS].bitcast(F32),
                    in_=tmp[:].rearrange("p (h f) -> p h f", h=HEADS),
                    axis=X, op=A.add)
                nc.vector.tensor_tensor(out=tmp[:], in0=h_ps[:],
                                        in1=ad1_sb[:], op=A.mult)
                adf = sp2.tile([P, HEADS], F32)
                nc.vector.tensor_reduce(
                    out=adf[:],
                    in_=tmp[:].rearrange("p (h f) -> p h f", h=HEADS),
                    axis=X, op=A.add)
                nc.vector.tensor_copy(out=adst_all[:, t, :], in_=adf[:])
                nc.vector.tensor_copy(out=t1t[:, 0:D1], in_=h_ps[:])
                nc.vector.memset(t1t[:, D1 + 2 * HEADS:ROW1], 0.0)
                nc.sync.dma_start(t1loc[t * P:(t + 1) * P, :], t1t[:])
            nc.gpsimd.collective_compute(
                "AllGather", A.bypass, replica_groups=rg,
                ins=[t1loc[:]], outs=[t1full[:]])

        # ---------------- Phase 1: layer-1 edge phase -----------------------
        with tc.tile_pool(name="p1g", bufs=3) as pg, \
             tc.tile_pool(name="p1u", bufs=2) as pu, \
             tc.tile_pool(name="p1o", bufs=2) as po, \
             tc.tile_pool(name="p1w", bufs=3) as pw, \
             tc.tile_pool(name="p1e", bufs=1) as pe, \
             tc.tile_pool(name="p1b", bufs=2) as pb, \
             tc.tile_pool(name="p1ps", bufs=2, space="PSUM") as pps, \
             tc.tile_pool(name="p1pa", bufs=2, space="PSUM") as ppa:
            b1_sb = pe.tile([P, D1], F32)
            nc.sync.dma_start(b1_sb[:], b1_d[:])
            w2_sb = pe.tile([P, 2, HID], BF16)
            nc.sync.dma_start(w2_sb[:], W2_d[:])
            as2_sb = pe.tile([P, HID], F32)
            nc.sync.dma_start(as2_sb[:], as2_d[:])
            ad2_sb = pe.tile([P, HID], F32)
            nc.sync.dma_start(ad2_sb[:], ad2_d[:])
            for t in range(NT):
                G = pg.tile([P, CPT, ROW1], BF16)
                if t < 3:
                    nc.vector.memset(G[:], 0.0)
                nc.gpsimd.dma_gather(
                    G[:], t1full[:], ixs_all[:, t * NIc:(t + 1) * NIc],
                    NI, NI, ROW1, single_packet=False)
                du = pu.tile([P, NI], U8)
                nc.sync.dma_start(du[:],
                                  du_d[0:1, t, :].partition_broadcast(P))
                ohT = po.tile([P, NI], BF16)
                nc.vector.tensor_scalar(out=ohT[:], in0=du[:],
                                        scalar1=iotac_sb[:, 0:1],
                                        scalar2=None, op0=A.is_equal)
                ae_ps = ppa.tile([P, CPT, HEADS], F32, space="PSUM")
                for c in range(CPT):
                    nc.tensor.matmul(ae_ps[:, c, :],
                                     lhsT=ohT[:, c * P:(c + 1) * P],
                                     rhs=adst_all[:, t, :],
                                     start=True, stop=True)
                oh = po.tile([P, CPT, P], BF16)
                nc.vector.tensor_tensor(
                    out=oh[:],
                    in0=iotab_sb[:].unsqueeze(1).to_broadcast([P, CPT, P]),
                    in1=dstl_sb[:, t * CPT:(t + 1) * CPT]
                        .unsqueeze(2).to_broadcast([P, CPT, P]),
                    op=A.is_equal)
                # alpha = a_src + a_dst + w*q ; p = max(exp(a), exp(0.2a))
                asr = pw.tile([P, CPT, HEADS], F32)
                nc.vector.tensor_tensor(
                    out=asr[:],
                    in0=wv_sb[:, t * CPT:(t + 1) * CPT].unsqueeze(2)
                        .to_broadcast([P, CPT, HEADS]),
                    in1=q1_sb[:].unsqueeze(1).to_broadcast([P, CPT, HEADS]),
                    op=A.mult)
                nc.vector.tensor_tensor(
                    out=asr[:], in0=asr[:],
                    in1=G[:, :, D1:D1 + 2 * HEADS].bitcast(F32), op=A.add)
                nc.vector.tensor_tensor(out=asr[:], in0=asr[:], in1=ae_ps[:],
                                        op=A.add)
                e2 = pw.tile([P, CPT, HEADS], F32)
                nc.scalar.activation(out=e2[:], in_=asr[:], func=ACT.Exp,
                                     scale=0.2)
                nc.scalar.activation(out=asr[:], in_=asr[:], func=ACT.Exp)
                pbf = G[:, :, D1:D1 + HEADS]   # bf16 p slot
                nc.vector.tensor_tensor(out=pbf, in0=asr[:], in1=e2[:],
                                        op=A.max)
                gm = G[:, :, 0:D1].rearrange("p c (h f) -> p c h f", h=HEADS)
                nc.vector.tensor_tensor(
                    out=gm, in0=gm,
                    in1=pbf.unsqueeze(3).to_broadcast([P, CPT, HEADS, HID]),
                    op=A.mult)
                acc = pps.tile([P, D1 + HEADS], F32, space="PSUM")
                for c in range(CPT):
                    nc.tensor.matmul(acc[:], lhsT=oh[:, c, :],
                                     rhs=G[:, c, 0:D1 + HEADS],
                                     start=(c == 0), stop=(c == CPT - 1))
                # epilogue: out1 = relu(acc/denom + b1)
                dn = pw.tile([P, HEADS], F32)
                nc.vector.tensor_scalar(out=dn[:], in0=acc[:, D1:D1 + HEADS],
                                        scalar1=1e-16, scalar2=None,
                                        op0=A.add)
                rc = pw.tile([P, HEADS], F32)
                nc.vector.reciprocal(rc[:], dn[:])
                ob = out1[:, t * D1:(t + 1) * D1]
                nc.vector.tensor_tensor(
                    out=ob.rearrange("p (h f) -> p h f", h=HEADS),
                    in0=acc[:, 0:D1].rearrange("p (h f) -> p h f", h=HEADS),
                    in1=rc[:].unsqueeze(2).to_broadcast([P, HEADS, HID]),
                    op=A.mult)
                nc.vector.tensor_tensor(out=ob, in0=ob, in1=b1_sb[:],
                                        op=A.add)
                nc.vector.tensor_scalar(out=ob, in0=ob, scalar1=0.0,
                                        scalar2=None, op0=A.max)
                # ---- layer-2 row for this tile (h2, a_src2, a_dst2) --------
                h2_ps = ppa.tile([P, HID], F32, space="PSUM")
                for k in range(2):
                    hT_ps = ppa.tile([P, P], BF16, space="PSUM")
                    nc.tensor.transpose(
                        hT_ps[:],
                        out1[:, t * D1 + k * P:t * D1 + (k + 1) * P],
                        identb[:])
                    hT = pb.tile([P, P], BF16)
                    nc.vector.tensor_copy(out=hT[:], in_=hT_ps[:])
                    nc.tensor.matmul(h2_ps[:], lhsT=hT[:],
                                     rhs=w2_sb[:, k, :],
                                     start=(k == 0), stop=(k == 1))
                t2t = pb.tile([P, ROW2], BF16)
                tmp = pb.tile([P, HID], F32)
                nc.vector.tensor_tensor(out=tmp[:], in0=h2_ps[:],
                                        in1=as2_sb[:], op=A.mult)
                nc.vector.tensor_reduce(out=t2t[:, HID:HID + 2].bitcast(F32),
                                        in_=tmp[:], axis=X, op=A.add)
                nc.vector.tensor_tensor(out=tmp[:], in0=h2_ps[:],
                                        in1=ad2_sb[:], op=A.mult)
                ad2f = pb.tile([P, 1], F32)
                nc.vector.tensor_reduce(out=ad2f[:], in_=tmp[:], axis=X,
                                        op=A.add)
                nc.vector.tensor_copy(out=adst2_all[:, t:t + 1], in_=ad2f[:])
                nc.vector.tensor_copy(out=t2t[:, 0:HID], in_=h2_ps[:])
                nc.vector.memset(t2t[:, HID + 2:ROW2], 0.0)
                nc.sync.dma_start(t2loc[t * P:(t + 1) * P, :], t2t[:])
            nc.gpsimd.collective_compute(
                "AllGather", A.bypass, replica_groups=rg,
                ins=[t2loc[:]], outs=[t2full[:]])

        # ---------------- Phase 3: layer-2 edge phase + pooling -------------
        with tc.tile_pool(name="p3g", bufs=4) as pg, \
             tc.tile_pool(name="p3u", bufs=2) as pu, \
             tc.tile_pool(name="p3o", bufs=3) as po, \
             tc.tile_pool(name="p3w", bufs=3) as pw, \
             tc.tile_pool(name="p3c", bufs=1) as pc, \
             tc.tile_pool(name="p3ps", bufs=2, space="PSUM") as pps, \
             tc.tile_pool(name="p3pa", bufs=2, space="PSUM") as ppa, \
             tc.tile_pool(name="p3pl", bufs=1, space="PSUM") as ppl:
            b2_sb = pc.tile([P, HID], F32)
            nc.sync.dma_start(b2_sb[:], b2_d[:])
            pool_ps = ppl.tile([GPC, HID + 1], F32, space="PSUM")
            for t in range(NT):
                G = pg.tile([P, CPT, ROW2], BF16)
                if t < 4:
                    nc.vector.memset(G[:], 0.0)
                nc.gpsimd.dma_gather(
                    G[:], t2full[:], ixs_all[:, t * NIc:(t + 1) * NIc],
                    NI, NI, ROW2, single_packet=False)
                du = pu.tile([P, NI], U8)
                nc.sync.dma_start(du[:],
                                  du_d[0:1, t, :].partition_broadcast(P))
                ohT = po.tile([P, NI], BF16)
                nc.vector.tensor_scalar(out=ohT[:], in0=du[:],
                                        scalar1=iotac_sb[:, 0:1],
                                        scalar2=None, op0=A.is_equal)
                ae_ps = ppa.tile([P, CPT, 1], F32, space="PSUM")
                for c in range(CPT):
                    nc.tensor.matmul(ae_ps[:, c, :],
                                     lhsT=ohT[:, c * P:(c + 1) * P],
                                     rhs=adst2_all[:, t:t + 1],
                                     start=True, stop=True)
                oh = po.tile([P, CPT, P], BF16)
                nc.vector.tensor_tensor(
                    out=oh[:],
                    in0=iotab_sb[:].unsqueeze(1).to_broadcast([P, CPT, P]),
                    in1=dstl_sb[:, t * CPT:(t + 1) * CPT]
                        .unsqueeze(2).to_broadcast([P, CPT, P]),
                    op=A.is_equal)
                asr = pw.tile([P, CPT, 1], F32)
                nc.vector.tensor_scalar(
                    out=asr[:], in0=wv_sb[:, t * CPT:(t + 1) * CPT]
                        .unsqueeze(2),
                    scalar1=q2_sb[:, 0:1], scalar2=None, op0=A.mult)
                nc.vector.tensor_tensor(
                    out=asr[:], in0=asr[:],
                    in1=G[:, :, HID:HID + 2].bitcast(F32), op=A.add)
                nc.vector.tensor_tensor(out=asr[:], in0=asr[:], in1=ae_ps[:],
                                        op=A.add)
                e2 = pw.tile([P, CPT, 1], F32)
                nc.scalar.activation(out=e2[:], in_=asr[:], func=ACT.Exp,
                                     scale=0.2)
                nc.scalar.activation(out=asr[:], in_=asr[:], func=ACT.Exp)
                pbf = G[:, :, HID:HID + 1]
                nc.vector.tensor_tensor(out=pbf, in0=asr[:], in1=e2[:],
                                        op=A.max)
                gm = G[:, :, 0:HID]
                nc.vector.tensor_tensor(
                    out=gm, in0=gm,
                    in1=pbf.to_broadcast([P, CPT, HID]), op=A.mult)
                acc = pps.tile([P, HID + 1], F32, space="PSUM")
                for c in range(CPT):
                    nc.tensor.matmul(acc[:], lhsT=oh[:, c, :],
                                     rhs=G[:, c, 0:HID + 1],
                                     start=(c == 0), stop=(c == CPT - 1))
                # epilogue: o2 = [relu(acc/denom + b2) | 1], pool matmul
                dn = pw.tile([P, 1], F32)
                nc.vector.tensor_scalar(out=dn[:], in0=acc[:, HID:HID + 1],
                                        scalar1=1e-16, scalar2=None,
                                        op0=A.add)
                rc = pw.tile([P, 1], F32)
                nc.vector.reciprocal(rc[:], dn[:])
                o2 = pw.tile([P, HID + 1], F32)
                nc.vector.tensor_scalar(out=o2[:, 0:HID], in0=acc[:, 0:HID],
                                        scalar1=rc[:, 0:1], scalar2=None,
                                        op0=A.mult)
                nc.vector.tensor_tensor(out=o2[:, 0:HID], in0=o2[:, 0:HID],
                                        in1=b2_sb[:], op=A.add)
                nc.vector.tensor_scalar(out=o2[:, 0:HID], in0=o2[:, 0:HID],
                                        scalar1=0.0, scalar2=None, op0=A.max)
                nc.vector.memset(o2[:, HID:HID + 1], 1.0)
                ohg = pw.tile([P, GPC], F32)
                nc.vector.tensor_scalar(
                    out=ohg[:], in0=iota_sb[:, 0:GPC],
                    scalar1=gl_sb[:, t:t + 1], scalar2=None, op0=A.is_equal)
                nc.tensor.matmul(pool_ps[:], lhsT=ohg[:], rhs=o2[:],
                                 start=(t == 0), stop=(t == NT - 1),
                                 skip_group_check=True)

            # ------------- Phase 4: pooled mean + FC ------------------------
            fcw_sb = pc.tile([HID, OUT], F32)
            nc.sync.dma_start(fcw_sb[:], fcw_d[:])
            fcb_sb = pc.tile([P, OUT], F32)
            nc.sync.dma_start(fcb_sb[:], fcb_d[:])
            cnt = pc.tile([GPC, 1], F32)
            nc.vector.tensor_scalar(out=cnt[:], in0=pool_ps[:, HID:HID + 1],
                                    scalar1=1.0, scalar2=None, op0=A.max)
            rcc = pc.tile([GPC, 1], F32)
            nc.vector.reciprocal(rcc[:], cnt[:])
            pooled = pc.tile([GPC, HID], F32)
            nc.vector.tensor_scalar(out=pooled[:], in0=pool_ps[:, 0:HID],
                                    scalar1=rcc[:, 0:1], scalar2=None,
                                    op0=A.mult)
            pT_ps = ppl.tile([HID, GPC], F32, space="PSUM")
            nc.tensor.transpose(pT_ps[:], pooled[:], identf[:GPC, :GPC])
            pT = pc.tile([HID, GPC], F32)
            nc.vector.tensor_copy(out=pT[:], in_=pT_ps[:])
            fc_ps = ppl.tile([GPC, OUT], F32, space="PSUM")
            nc.tensor.matmul(fc_ps[:], lhsT=pT[:], rhs=fcw_sb[:],
                             start=True, stop=True)
            res = pc.tile([GPC, OUT], F32)
            nc.vector.tensor_tensor(out=res[:], in0=fc_ps[:],
                                    in1=fcb_sb[:GPC, :], op=A.add)
            nc.sync.dma_start(out_d[:], res[:])

    nc.compile()
    return nc


# ---------------------------------------------------------------------------
# Entry point.
# ---------------------------------------------------------------------------
def run(inputs, cfg, **run_kwargs):
    in_maps, meta = prepare(inputs, cfg)
    nc = build(meta)
    res = run_bass_kernel_spmd(nc, in_maps, core_ids=list(range(NCORES)),
                               **run_kwargs)
    out = np.concatenate([res.results[c]["out"] for c in range(NCORES)],
                         axis=0)
    return np.asarray(out, np.float32), res


def kernel(**inputs) -> np.ndarray:
    out, _ = run(inputs, FULL_CFG)
    return out
